# revision 1
# baseline (speedup 1.0000x reference)
"""Trainium2 Bass SPMD kernel: 16-head MHA (B=4, S=2048, D=1024), fp32.

Sharding: 8 cores = 4 batches x 2 head-groups (8 heads each). Host pre-
transposes activations to [D, S] and pre-slices/transposes weights, so the
device never transposes anything:

  - Q/K projections produce QT/KT in [d_local, S] layout (head dim on
    partitions) which directly feeds the scores matmul.
  - Scores are computed transposed ([t, s] in PSUM), exp'd on ScalarE
    (scale=1/8 folded in, no max-subtraction: scores*0.125 max ~10, exp
    ~3e4, fine in fp32), written to SBUF as fp32r.
  - V is produced in natural [t, d] layout with an appended ones column, so
    the PV matmul yields both the unnormalized output (rows 0..63) and the
    softmax denominator (row 64) in one pass.
  - Normalization: reciprocal of the denominator row + K=1 ones-matmul to
    broadcast it across partitions + one DVE multiply.
  - O-projection contracts attn^T [d_local, s] tiles against w_o columns;
    per-core partial outputs are summed (+b_o) on the host.

All matmuls run in float32r (full PE rate at N=512, ~1.6e-4 rel err).
"""
import numpy as np

import concourse.bass as bass
import concourse.mybir as mybir
from concourse.tile import TileContext
from concourse.bass_utils import run_bass_kernel_spmd

F32 = mybir.dt.float32
F32R = mybir.dt.float32r
AF = mybir.ActivationFunctionType

B, S, D = 4, 2048, 1024
H, DH = 16, 64
HL = 8        # heads per core
DL = HL * DH  # 512 local model dims
P = 128
SC = 512      # s-chunk width
NCH = S // SC  # 4 s-chunks
ND = D // P    # 8 contraction subtiles for D
NM = DL // P   # 4 m-tiles of local outputs
NT = S // P    # 16 t-tiles

_MAX_WAITS = 1


def _split_excess_waits(nc, max_waits=_MAX_WAITS):
    """walrus here rejects >1 sync-wait per instruction; spill extras onto
    same-engine NoOps inserted before the instruction."""
    f = nc.m.functions[0]
    n = 0
    for bb in f.blocks:
        changed = False
        out = []
        for inst in bb.instructions:
            si = inst.sync_info
            if si is not None and len(si.on_wait) > max_waits:
                waits = list(si.on_wait)
                keep = waits[-max_waits:]
                spill = waits[:-max_waits]
                for i in range(0, len(spill), max_waits):
                    nop = mybir.InstNoOp(name=f"WSPILL-{n}", ins=[], outs=[])
                    n += 1
                    nop.engine = inst.engine
                    nop.sync_info = mybir.SyncInfo(
                        on_wait=spill[i : i + max_waits], on_update=[]
                    )
                    nc.register_instruction(nop, overwrite=True)
                    out.append(nop)
                inst.sync_info = mybir.SyncInfo(
                    on_wait=keep, on_update=list(si.on_update)
                )
                changed = True
            out.append(inst)
        if changed:
            bb.instructions = out
    return n


def build():
    nc = bass.Bass()
    xq = nc.dram_tensor("xq", [D, S], F32R, kind="ExternalInput")
    xk = nc.dram_tensor("xk", [D, S], F32R, kind="ExternalInput")
    xv = nc.dram_tensor("xv", [D, S], F32R, kind="ExternalInput")
    wq = nc.dram_tensor("wq", [D, DL], F32R, kind="ExternalInput")
    wk = nc.dram_tensor("wk", [D, DL], F32R, kind="ExternalInput")
    wv = nc.dram_tensor("wv", [D, DL], F32R, kind="ExternalInput")
    wo = nc.dram_tensor("wo", [DL, D], F32R, kind="ExternalInput")
    bq = nc.dram_tensor("bq", [DL], F32, kind="ExternalInput")
    bk = nc.dram_tensor("bk", [DL], F32, kind="ExternalInput")
    bv = nc.dram_tensor("bv", [DL], F32R, kind="ExternalInput")
    out = nc.dram_tensor("out", [S, D], F32, kind="ExternalOutput")

    xq_r = xq.rearrange("(o p) s -> p o s", p=P)
    xk_r = xk.rearrange("(o p) s -> p o s", p=P)
    xv_r = xv.rearrange("(o p) s -> p o s", p=P)
    wq_r = wq.rearrange("(o p) m -> p o m", p=P)
    wk_r = wk.rearrange("(o p) m -> p o m", p=P)
    wv_r = wv.rearrange("(o p) m -> p o m", p=P)
    wo_r = wo.rearrange("(o p) n -> p o n", p=P)
    out_r = out.rearrange("(so p) n -> p so n", p=P)

    with TileContext(nc) as tc:
        with (
            tc.tile_pool(name="persist", bufs=1) as persist,
            tc.tile_pool(name="wpool", bufs=1) as wpool,
            tc.tile_pool(name="xpool", bufs=3) as xpool,
            tc.tile_pool(name="expp", bufs=2) as expp,
            tc.tile_pool(name="attnp", bufs=2) as attnp,
            tc.tile_pool(name="osb", bufs=2) as osbp,
            tc.tile_pool(name="nrm", bufs=2) as nrm,
            tc.tile_pool(name="ps_big", bufs=1, space="PSUM") as ps_big,
            tc.tile_pool(name="ps_pv", bufs=1, space="PSUM") as ps_pv,
            tc.tile_pool(name="ps_sm", bufs=2, space="PSUM") as ps_sm,
        ):
            qt = persist.tile([P, NM, S], F32R, tag="qt")
            kt = persist.tile([P, NM, S], F32R, tag="kt")
            vaug = persist.tile([P, NT, HL, 66], F32R, tag="vaug")
            wq_t = persist.tile([P, ND, DL], F32R, tag="wq")
            ones_f = persist.tile([P, P], F32, tag="ones_f")
            ones_r = persist.tile([P, P], F32R, tag="ones_r")
            bq_sb = persist.tile([P, NM], F32, tag="bq")
            bk_sb = persist.tile([P, NM], F32, tag="bk")
            bv_t = persist.tile([P, DL], F32R, tag="bv_t")
            bv_bc = persist.tile([P, DL], F32, tag="bv_bc")

            # ---- constants / biases ----
            nc.vector.memset(ones_f[:], 1.0)
            nc.vector.tensor_copy(ones_r[:], ones_f[:])
            nc.sync.dma_start(bq_sb[:], bq.rearrange("(o p) -> p o", p=P))
            nc.sync.dma_start(bk_sb[:], bk.rearrange("(o p) -> p o", p=P))
            nc.sync.dma_start(bv_t[0:1, :], bv[None, :])
            ps = ps_sm.tile([P, SC], F32, tag="sm")
            nc.tensor.matmul(ps[:], ones_r[0:1, 0:P], bv_t[0:1, :], start=True,
                             stop=True)
            nc.vector.tensor_copy(bv_bc[:], ps[:])
            # ones columns of V_aug
            of = ones_f[:, 0:NT * HL].rearrange("p (a b) -> p a b", a=NT)
            nc.vector.tensor_copy(vaug[:, :, :, 0:1], of[:, :, :, None])
            nc.vector.tensor_copy(vaug[:, :, :, 65:66], of[:, :, :, None])

            def proj_chunk(dst, w_tile, x_r, c, bias_sb):
                xa = xpool.tile([P, ND // 2, SC], F32R, tag="x")
                xb = xpool.tile([P, ND // 2, SC], F32R, tag="x")
                nc.sync.dma_start(xa[:], x_r[:, 0 : ND // 2, c * SC : (c + 1) * SC])
                nc.sync.dma_start(xb[:], x_r[:, ND // 2 : ND, c * SC : (c + 1) * SC])
                halves = (xa, xb)
                for m in range(NM):
                    psm = ps_sm.tile([P, SC], F32, tag="sm")
                    for k in range(ND):
                        nc.tensor.matmul(
                            psm[:],
                            w_tile[:, k, m * P : (m + 1) * P],
                            halves[k // 4][:, k % 4, :],
                            start=(k == 0),
                            stop=(k == ND - 1),
                        )
                    nc.vector.tensor_add(
                        dst[:, m, c * SC : (c + 1) * SC],
                        psm[:],
                        bias_sb[:, m : m + 1].to_broadcast((P, SC)),
                    )

            def vproj_chunk(wv_t, c):
                xa = xpool.tile([P, ND // 2, SC], F32R, tag="x")
                xb = xpool.tile([P, ND // 2, SC], F32R, tag="x")
                nc.sync.dma_start(xa[:], xv_r[:, 0 : ND // 2, c * SC : (c + 1) * SC])
                nc.sync.dma_start(xb[:], xv_r[:, ND // 2 : ND, c * SC : (c + 1) * SC])
                halves = (xa, xb)
                for i in range(4):
                    t_o = c * 4 + i
                    psm = ps_sm.tile([P, SC], F32, tag="sm")
                    for k in range(ND):
                        nc.tensor.matmul(
                            psm[:],
                            halves[k // 4][:, k % 4, i * P : (i + 1) * P],
                            wv_t[:, k, :],
                            start=(k == 0),
                            stop=(k == ND - 1),
                        )
                    for h in range(HL):
                        nc.vector.tensor_add(
                            vaug[:, t_o, h, 1:65],
                            psm[:, h * DH : (h + 1) * DH],
                            bv_bc[:, h * DH : (h + 1) * DH],
                        )

            def normalize_pre(pv):
                """Consume the PV psum right away on DVE (frees the psum slot):
                copy unnormalized rows, reciprocal of the denominator row."""
                raw = nrm.tile([P, SC], F32, tag="raw")
                rec = nrm.tile([P, SC], F32R, tag="rec")
                nc.vector.tensor_copy(raw[0:64, :], pv[0:64, :])
                nc.vector.tensor_copy(raw[64:65, :], pv[64:65, :])
                with nc.allow_low_precision(reason="fp32r recip for matmul rhs"):
                    nc.vector.reciprocal(rec[64:65, :], raw[64:65, :])
                return raw, rec

            def normalize_post(raw, rec, dst_lo, dst_hi_dma):
                """Broadcast 1/denom across partitions (K=1 matmul) and apply."""
                bc = ps_sm.tile([P, SC], F32, tag="sm")
                nc.tensor.matmul(bc[0:64, :], ones_r[64:65, 0:64], rec[64:65, :],
                                 start=True, stop=True)
                if dst_hi_dma is None:
                    nc.vector.tensor_mul(dst_lo, bc[0:64, :], raw[0:64, :])
                else:
                    tmp = nrm.tile([P, SC], F32R, tag="tmp")
                    nc.vector.tensor_mul(tmp[0:64, :], bc[0:64, :], raw[0:64, :])
                    nc.sync.dma_start(dst_hi_dma, tmp[0:64, :])

            # ---- projections (prefix) ----
            nc.sync.dma_start(wq_t[:], wq_r[:])
            proj_chunk(qt, wq_t, xq_r, 0, bq_sb)
            wk_t = wpool.tile([P, ND, DL], F32R, tag="w")
            nc.sync.dma_start(wk_t[:], wk_r[:])
            for c in range(NCH):
                proj_chunk(kt, wk_t, xk_r, c, bk_sb)
            wv_t = wpool.tile([P, ND, DL], F32R, tag="w")
            nc.sync.dma_start(wv_t[:], wv_r[:])
            for c in range(NCH):
                vproj_chunk(wv_t, c)
            wo_t = wpool.tile([P, NM, D], F32R, tag="w")
            nc.sync.dma_start(wo_t[:], wo_r[:])

            # ---- attention + o-proj, per s-chunk ----
            def oproj_chunk(c, attn_t):
                for st in range(4):
                    for n in range(2):
                        psm = ps_sm.tile([P, SC], F32, tag="sm")
                        for do in range(NM):
                            nc.tensor.matmul(
                                psm[:],
                                attn_t[:, do, st * P : (st + 1) * P],
                                wo_t[:, do, n * SC : (n + 1) * SC],
                                start=(do == 0),
                                stop=(do == NM - 1),
                            )
                        ob = osbp.tile([P, SC], F32, tag="ob")
                        nc.vector.tensor_copy(ob[:], psm[:])
                        nc.sync.dma_start(
                            out_r[:, c * 4 + st, n * SC : (n + 1) * SC], ob[:]
                        )

            pending_oproj = None
            for c in range(NCH):
                attn_t = attnp.tile([P, NM, SC], F32R, tag="attn")
                cs = slice(c * SC, (c + 1) * SC)
                deferred = None
                for p in range(NM):
                    pv_e = ps_pv.tile([P, SC], F32, tag="pv_e")
                    pv_o = ps_pv.tile([P, SC], F32, tag="pv_o")

                    def emit_pv(ex, ta, tb, p=p, pv_e=pv_e, pv_o=pv_o):
                        for j, (t, hh) in enumerate(
                            [(ta, 2 * p), (tb, 2 * p), (ta, 2 * p + 1),
                             (tb, 2 * p + 1)]
                        ):
                            pv = pv_e if j < 2 else pv_o
                            nc.tensor.matmul(
                                pv[0:65, :],
                                vaug[:, t, hh, 1:66],
                                ex[:, j, :],
                                start=(t == 0),
                                stop=(t == NT - 1),
                            )

                    pend = None
                    for g in range(NT // 2):
                        ta, tb = 2 * g, 2 * g + 1
                        big = ps_big.tile([P, 4, SC], F32, tag="big")
                        for j, (t, lo) in enumerate(
                            [(ta, 0), (tb, 0), (ta, 64), (tb, 64)]
                        ):
                            nc.tensor.matmul(
                                big[:, j, :],
                                kt[lo : lo + 64, p, t * P : (t + 1) * P],
                                qt[lo : lo + 64, p, cs],
                                start=True,
                                stop=True,
                                tile_position=(lo, 0),
                            )
                        ex = expp.tile([P, 4, SC], F32R, tag="ex")
                        nc.scalar.activation(ex[:], big[:], AF.Exp, scale=0.125)
                        if g == 4 and deferred is not None:
                            deferred()
                            deferred = None
                        if g == 5 and p == 0 and pending_oproj is not None:
                            pending_oproj()
                            pending_oproj = None
                        if g == 2 and p == 3 and c < NCH - 1:
                            proj_chunk(qt, wq_t, xq_r, c + 1, bq_sb)
                        if pend is not None:
                            emit_pv(*pend)
                        pend = (ex, ta, tb)
                    emit_pv(*pend)
                    raw_e, rec_e = normalize_pre(pv_e)
                    raw_o, rec_o = normalize_pre(pv_o)

                    def make_deferred(raw_e=raw_e, rec_e=rec_e, raw_o=raw_o,
                                      rec_o=rec_o, p=p, attn_t=attn_t):
                        def run():
                            normalize_post(raw_e, rec_e, attn_t[0:64, p, :], None)
                            normalize_post(raw_o, rec_o, None,
                                           attn_t[64:128, p, :])
                        return run

                    deferred = make_deferred()
                deferred()

                def make_oproj(c=c, attn_t=attn_t):
                    return lambda: oproj_chunk(c, attn_t)

                pending_oproj = make_oproj()
            pending_oproj()

    _split_excess_waits(nc)
    return nc


_CACHE = {}


def _get_nc():
    if "nc" not in _CACHE:
        _CACHE["nc"] = build()
    return _CACHE["nc"]


def _f32(x):
    return np.asarray(x).astype(np.float32, copy=False)


def _prep_core_inputs(c, q, k, v, w_q, b_q, w_k, b_k, w_v, b_v, w_o, b_o):
    b, hg = c // 2, c % 2
    hs = slice(hg * DL, hg * DL + DL)
    return {
        "xq": np.ascontiguousarray(q[b].T),
        "xk": np.ascontiguousarray(k[b].T),
        "xv": np.ascontiguousarray(v[b].T),
        "wq": np.ascontiguousarray(w_q[hs, :].T),
        "wk": np.ascontiguousarray(w_k[hs, :].T),
        "wv": np.ascontiguousarray(w_v[hs, :].T),
        "wo": np.ascontiguousarray(w_o[:, hs].T),
        "bq": np.ascontiguousarray(b_q[hs]),
        "bk": np.ascontiguousarray(b_k[hs]),
        "bv": np.ascontiguousarray(b_v[hs]),
    }


def kernel(q, k, v, w_q, b_q, w_k, b_k, w_v, b_v, w_o, b_o):
    q, k, v = _f32(q), _f32(k), _f32(v)
    w_q, b_q = _f32(w_q), _f32(b_q)
    w_k, b_k = _f32(w_k), _f32(b_k)
    w_v, b_v = _f32(w_v), _f32(b_v)
    w_o, b_o = _f32(w_o), _f32(b_o)

    nc = _get_nc()
    in_maps = [
        _prep_core_inputs(c, q, k, v, w_q, b_q, w_k, b_k, w_v, b_v, w_o, b_o)
        for c in range(8)
    ]
    res = run_bass_kernel_spmd(nc, in_maps, core_ids=list(range(8)))
    out = np.empty((B, S, D), np.float32)
    for b in range(B):
        out[b] = res.results[2 * b]["out"] + res.results[2 * b + 1]["out"] + b_o
    return out



# revision 6
# speedup vs baseline: 1.1045x; 1.1045x over previous
"""Trainium2 Bass SPMD kernel: 16-head MHA (B=4, S=2048, D=1024), fp32 in/out.

Sharding: 8 cores = 4 batches x 2 head-groups (8 heads each). Host pre-
transposes activations to [D, S], pre-slices/transposes weights, and casts
everything (except biases) to bf16 — fp32 matmuls stream at ~1.8 cyc/col on
TRN2 while bf16 streams at 1 cyc/col, and the rel-err budget (2e-2) has
~4x headroom over the measured all-bf16 error (~6e-3).

Pipeline (per s-chunk of 512, per head-pair p):
  - t-loop over 16 key tiles: two K=64 scores matmuls (row-tiled at
    partitions 0/64, run concurrently on the PE) write a 2-bank PSUM tile;
    ScalarE exps it (scale=1/8 folded, no max-subtraction) into bf16 SBUF;
    the PV matmuls for t-1 (65-row V_aug with an appended ones column
    yielding output + softmax denominator) accumulate into 2 PSUM banks.
  - Scores PSUM is double-buffered (2+2 banks) so scores(t+1) overlaps
    exp(t): the ScalarE exp stream (~266us) is the critical engine; all
    other work (o-proj of the previous chunk, q-proj of the next chunk,
    softmax normalization) is sliced into small pieces and slotted into
    the t-loop so the PE rides under the ScalarE ceiling.
  - Normalization: DVE reciprocal_approx_fast on the denominator row
    (~5x faster than exact reciprocal), K=1 ones-matmul broadcast across
    partitions, one DVE multiply into bf16 attn tiles.
  - O-projection contracts attn^T [d_local, s] against w_o columns;
    per-core partial outputs are summed (+b_o) on the host.
"""
import numpy as np
from ml_dtypes import bfloat16

import concourse.bass as bass
import concourse.mybir as mybir
from concourse.tile import TileContext
from concourse.bass_utils import run_bass_kernel_spmd

F32 = mybir.dt.float32
BF16 = mybir.dt.bfloat16
AF = mybir.ActivationFunctionType

B, S, D = 4, 2048, 1024
H, DH = 16, 64
HL = 8        # heads per core
DL = HL * DH  # 512 local model dims
P = 128
SC = 512      # s-chunk width
NCH = S // SC  # 4 s-chunks
ND = D // P    # 8 contraction subtiles for D
NM = DL // P   # 4 m-tiles of local outputs
NT = S // P    # 16 t-tiles

_MAX_WAITS = 1


def _split_excess_waits(nc, max_waits=_MAX_WAITS):
    """walrus here rejects >1 sync-wait per instruction; spill extras onto
    same-engine NoOps inserted before the instruction."""
    f = nc.m.functions[0]
    n = 0
    for bb in f.blocks:
        changed = False
        out = []
        for inst in bb.instructions:
            si = inst.sync_info
            if si is not None and len(si.on_wait) > max_waits:
                waits = list(si.on_wait)
                keep = waits[-max_waits:]
                spill = waits[:-max_waits]
                for i in range(0, len(spill), max_waits):
                    nop = mybir.InstNoOp(name=f"WSPILL-{n}", ins=[], outs=[])
                    n += 1
                    nop.engine = inst.engine
                    nop.sync_info = mybir.SyncInfo(
                        on_wait=spill[i : i + max_waits], on_update=[]
                    )
                    nc.register_instruction(nop, overwrite=True)
                    out.append(nop)
                inst.sync_info = mybir.SyncInfo(
                    on_wait=keep, on_update=list(si.on_update)
                )
                changed = True
            out.append(inst)
        if changed:
            bb.instructions = out
    return n


def build():
    nc = bass.Bass()
    xq = nc.dram_tensor("xq", [D, S], BF16, kind="ExternalInput")
    xk = nc.dram_tensor("xk", [D, S], BF16, kind="ExternalInput")
    xv = nc.dram_tensor("xv", [D, S], BF16, kind="ExternalInput")
    wq = nc.dram_tensor("wq", [D, DL], BF16, kind="ExternalInput")
    wk = nc.dram_tensor("wk", [D, DL], BF16, kind="ExternalInput")
    wv = nc.dram_tensor("wv", [D, DL], BF16, kind="ExternalInput")
    wo = nc.dram_tensor("wo", [DL, D], BF16, kind="ExternalInput")
    bq = nc.dram_tensor("bq", [DL], F32, kind="ExternalInput")
    bk = nc.dram_tensor("bk", [DL], F32, kind="ExternalInput")
    bv = nc.dram_tensor("bv", [DL], F32, kind="ExternalInput")
    out = nc.dram_tensor("out", [S, D], F32, kind="ExternalOutput")

    xq_r = xq.rearrange("(o p) s -> p o s", p=P)
    xk_r = xk.rearrange("(o p) s -> p o s", p=P)
    xv_r = xv.rearrange("(o p) s -> p o s", p=P)
    wq_r = wq.rearrange("(o p) m -> p o m", p=P)
    wk_r = wk.rearrange("(o p) m -> p o m", p=P)
    wv_r = wv.rearrange("(o p) m -> p o m", p=P)
    wo_r = wo.rearrange("(o p) n -> p o n", p=P)
    out_r = out.rearrange("(so p) n -> p so n", p=P)

    with TileContext(nc) as tc:
        with (
            tc.tile_pool(name="persist", bufs=1) as persist,
            tc.tile_pool(name="wpool", bufs=1) as wpool,
            tc.tile_pool(name="xpool", bufs=3) as xpool,
            tc.tile_pool(name="expp", bufs=3) as expp,
            tc.tile_pool(name="attnp", bufs=2) as attnp,
            tc.tile_pool(name="osb", bufs=2) as osbp,
            tc.tile_pool(name="nrm", bufs=2) as nrm,
            tc.tile_pool(name="ps_sc", bufs=2, space="PSUM") as ps_sc,
            tc.tile_pool(name="ps_pv", bufs=1, space="PSUM") as ps_pv,
            tc.tile_pool(name="ps_sm", bufs=2, space="PSUM") as ps_sm,
        ):
            qt = persist.tile([P, NM, S], BF16, tag="qt")
            kt = persist.tile([P, NM, S], BF16, tag="kt")
            vaug = persist.tile([P, NT, HL, 66], BF16, tag="vaug")
            wq_t = persist.tile([P, ND, DL], BF16, tag="wq")
            ones_f = persist.tile([P, P], F32, tag="ones_f")
            bq_sb = persist.tile([P, NM], F32, tag="bq")
            bk_sb = persist.tile([P, NM], F32, tag="bk")
            bv_t = persist.tile([P, DL], F32, tag="bv_t")
            bv_bc = persist.tile([P, DL], F32, tag="bv_bc")

            # ---- constants / biases ----
            nc.vector.memset(ones_f[:], 1.0)
            nc.sync.dma_start(bq_sb[:], bq.rearrange("(o p) -> p o", p=P))
            nc.sync.dma_start(bk_sb[:], bk.rearrange("(o p) -> p o", p=P))
            nc.sync.dma_start(bv_t[0:1, :], bv[None, :])
            ps = ps_sm.tile([P, SC], F32, tag="sm")
            nc.tensor.matmul(ps[:], ones_f[0:1, 0:P], bv_t[0:1, :], start=True,
                             stop=True)
            nc.vector.tensor_copy(bv_bc[:], ps[:])
            # ones column of V_aug (output row 64 = softmax denominator)
            of = ones_f[:, 0:NT * HL].rearrange("p (a b) -> p a b", a=NT)
            nc.vector.tensor_copy(vaug[:, :, :, 65:66], of[:, :, :, None])

            def proj_dma(x_r, c):
                xa = xpool.tile([P, ND // 2, SC], BF16, tag="x")
                xb = xpool.tile([P, ND // 2, SC], BF16, tag="x")
                nc.sync.dma_start(
                    xa[:], x_r[:, 0 : ND // 2, c * SC : (c + 1) * SC])
                nc.sync.dma_start(
                    xb[:], x_r[:, ND // 2 : ND, c * SC : (c + 1) * SC])
                return (xa, xb)

            def proj_chunk(dst, w_tile, x_r, c, bias_sb, m_tiles=range(NM),
                           halves=None):
                """Project one 512-col s-chunk for the given m-tiles."""
                if halves is None:
                    halves = proj_dma(x_r, c)
                for m in m_tiles:
                    psm = ps_sm.tile([P, SC], F32, tag="sm")
                    for k in range(ND):
                        nc.tensor.matmul(
                            psm[:],
                            w_tile[:, k, m * P : (m + 1) * P],
                            halves[k // 4][:, k % 4, :],
                            start=(k == 0),
                            stop=(k == ND - 1),
                        )
                    nc.vector.tensor_add(
                        dst[:, m, c * SC : (c + 1) * SC],
                        psm[:],
                        bias_sb[:, m : m + 1].to_broadcast((P, SC)),
                    )

            def vproj_chunk(wv_t, c):
                halves = proj_dma(xv_r, c)
                for i in range(4):
                    t_o = c * 4 + i
                    psm = ps_sm.tile([P, SC], F32, tag="sm")
                    for k in range(ND):
                        nc.tensor.matmul(
                            psm[:],
                            halves[k // 4][:, k % 4, i * P : (i + 1) * P],
                            wv_t[:, k, :],
                            start=(k == 0),
                            stop=(k == ND - 1),
                        )
                    nc.vector.tensor_add(
                        vaug[:, t_o, :, 1:65],
                        psm[:].rearrange("p (h d) -> p h d", h=HL),
                        bv_bc[:].rearrange("p (h d) -> p h d", h=HL),
                    )

            # ---- projections (prefix) ----
            nc.sync.dma_start(wq_t[:], wq_r[:])
            proj_chunk(qt, wq_t, xq_r, 0, bq_sb)
            wk_t = wpool.tile([P, ND, DL], BF16, tag="w")
            nc.sync.dma_start(wk_t[:], wk_r[:])
            for c in range(NCH):
                proj_chunk(kt, wk_t, xk_r, c, bk_sb)
            wv_t = wpool.tile([P, ND, DL], BF16, tag="w")
            nc.sync.dma_start(wv_t[:], wv_r[:])
            for c in range(NCH):
                vproj_chunk(wv_t, c)
            wo_t = wpool.tile([P, NM, D], BF16, tag="w")
            nc.sync.dma_start(wo_t[:], wo_r[:])

            # ---- attention + o-proj, per s-chunk ----
            def oproj_piece(c, attn_t, st, n):
                """One [128 s x 512 n] output tile of the o-projection."""
                psm = ps_sm.tile([P, SC], F32, tag="sm")
                for do in range(NM):
                    nc.tensor.matmul(
                        psm[:],
                        attn_t[:, do, st * P : (st + 1) * P],
                        wo_t[:, do, n * SC : (n + 1) * SC],
                        start=(do == 0),
                        stop=(do == NM - 1),
                    )
                ob = osbp.tile([P, SC], F32, tag="ob")
                nc.vector.tensor_copy(ob[:], psm[:])
                nc.sync.dma_start(
                    out_r[:, c * 4 + st, n * SC : (n + 1) * SC], ob[:]
                )

            def normalize_pre(pv):
                """Consume the PV psum right away on DVE (frees the psum slot):
                copy unnormalized rows, fast-reciprocal of the denominator."""
                raw = nrm.tile([P, SC], F32, tag="raw")
                rec = nrm.tile([P, SC], F32, tag="rec")
                nc.vector.tensor_copy(raw[0:65, :], pv[0:65, :])
                nc.vector.reciprocal(rec[64:65, :], raw[64:65, :])
                return raw, rec

            def normalize_post(raw, rec, dst):
                """Broadcast 1/denom across partitions (K=1 matmul), apply."""
                bc = ps_sm.tile([P, SC], F32, tag="sm")
                nc.tensor.matmul(bc[0:64, :], ones_f[64:65, 0:64], rec[64:65, :],
                                 start=True, stop=True)
                nc.vector.tensor_mul(dst, bc[0:64, :], raw[0:64, :])

            pending_norm = None   # (raw_e, rec_e, raw_o, rec_o, p, attn_t)
            pending_oproj = None  # (c, attn_t)
            for c in range(NCH):
                attn_t = attnp.tile([P, NM, SC], BF16, tag="attn")
                cs = slice(c * SC, (c + 1) * SC)
                for p in range(NM):
                    pv_e = ps_pv.tile([P, SC], F32, tag="pv_e")
                    pv_o = ps_pv.tile([P, SC], F32, tag="pv_o")
                    pend = None

                    def emit_pv(ex, t, p=p, pv_e=pv_e, pv_o=pv_o):
                        for j, (pv, hh) in enumerate(
                            [(pv_e, 2 * p), (pv_o, 2 * p + 1)]
                        ):
                            nc.tensor.matmul(
                                pv[0:65, :],
                                vaug[:, t, hh, 1:66],
                                ex[:, j, :],
                                start=(t == 0),
                                stop=(t == NT - 1),
                            )

                    for t in range(NT):
                        sct = ps_sc.tile([P, 2, SC], F32, tag="sc")
                        tw = slice(t * P, (t + 1) * P)
                        nc.tensor.matmul(
                            sct[:, 0, :], kt[0:64, p, tw], qt[0:64, p, cs],
                            start=True, stop=True, tile_position=(0, 0),
                        )
                        nc.tensor.matmul(
                            sct[:, 1, :], kt[64:128, p, tw], qt[64:128, p, cs],
                            start=True, stop=True, tile_position=(64, 0),
                        )
                        ex = expp.tile([P, 2, SC], BF16, tag="ex")
                        nc.scalar.activation(ex[:], sct[:], AF.Exp, scale=0.125)
                        if pend is not None:
                            emit_pv(*pend)
                        pend = (ex, t)

                        # -- interleaved off-ceiling work --
                        if t == 2 and pending_norm is not None:
                            raw_e, rec_e, raw_o, rec_o, pp, at = pending_norm
                            normalize_post(raw_e, rec_e, at[0:64, pp, :])
                            normalize_post(raw_o, rec_o, at[64:128, pp, :])
                            pending_norm = None
                        if p == 0 and pending_oproj is not None and \
                                t in (4, 6, 8, 10, 12, 14):
                            oc, oat = pending_oproj
                            i = (t - 4) // 2
                            oproj_piece(oc, oat, i // 2, i % 2)
                        if p == 1 and pending_oproj is not None and \
                                t in (2, 6):
                            oc, oat = pending_oproj
                            i = 6 + (t - 2) // 4
                            oproj_piece(oc, oat, i // 2, i % 2)
                            if i == 7:
                                pending_oproj = None
                        if p == 2 and c < NCH - 1 and t == 8:
                            next_halves = proj_dma(xq_r, c + 1)
                        if p == 3 and c < NCH - 1 and t in (2, 6, 10, 14):
                            proj_chunk(qt, wq_t, xq_r, c + 1, bq_sb,
                                       m_tiles=[(t - 2) // 4],
                                       halves=next_halves)
                    emit_pv(*pend)
                    raw_e, rec_e = normalize_pre(pv_e)
                    raw_o, rec_o = normalize_pre(pv_o)
                    pending_norm = (raw_e, rec_e, raw_o, rec_o, p, attn_t)

                # last head-pair of the chunk: normalize immediately so the
                # o-projection of this chunk can start early next chunk
                raw_e, rec_e, raw_o, rec_o, pp, at = pending_norm
                normalize_post(raw_e, rec_e, at[0:64, pp, :])
                normalize_post(raw_o, rec_o, at[64:128, pp, :])
                pending_norm = None
                pending_oproj = (c, attn_t)

            # drain: o-projection of the last chunk
            oc, oat = pending_oproj
            for st in range(4):
                for n in range(2):
                    oproj_piece(oc, oat, st, n)

    _split_excess_waits(nc)
    return nc


_CACHE = {}


def _get_nc():
    if "nc" not in _CACHE:
        _CACHE["nc"] = build()
    return _CACHE["nc"]


def _f32(x):
    return np.asarray(x).astype(np.float32, copy=False)


def _bf16(x):
    return np.ascontiguousarray(x.astype(bfloat16))


def _prep_core_inputs(c, q, k, v, w_q, b_q, w_k, b_k, w_v, b_v, w_o, b_o):
    b, hg = c // 2, c % 2
    hs = slice(hg * DL, hg * DL + DL)
    return {
        "xq": _bf16(q[b].T),
        "xk": _bf16(k[b].T),
        "xv": _bf16(v[b].T),
        "wq": _bf16(w_q[hs, :].T),
        "wk": _bf16(w_k[hs, :].T),
        "wv": _bf16(w_v[hs, :].T),
        "wo": _bf16(w_o[:, hs].T),
        "bq": np.ascontiguousarray(b_q[hs]),
        "bk": np.ascontiguousarray(b_k[hs]),
        "bv": np.ascontiguousarray(b_v[hs]),
    }


def kernel(q, k, v, w_q, b_q, w_k, b_k, w_v, b_v, w_o, b_o):
    q, k, v = _f32(q), _f32(k), _f32(v)
    w_q, b_q = _f32(w_q), _f32(b_q)
    w_k, b_k = _f32(w_k), _f32(b_k)
    w_v, b_v = _f32(w_v), _f32(b_v)
    w_o, b_o = _f32(w_o), _f32(b_o)

    nc = _get_nc()
    in_maps = [
        _prep_core_inputs(c, q, k, v, w_q, b_q, w_k, b_k, w_v, b_v, w_o, b_o)
        for c in range(8)
    ]
    res = run_bass_kernel_spmd(nc, in_maps, core_ids=list(range(8)))
    out = np.empty((B, S, D), np.float32)
    for b in range(B):
        out[b] = res.results[2 * b]["out"] + res.results[2 * b + 1]["out"] + b_o
    return out


# revision 8
# speedup vs baseline: 1.1114x; 1.0062x over previous
"""Trainium2 Bass SPMD kernel: 16-head MHA (B=4, S=2048, D=1024), fp32 in/out.

Sharding: 8 cores = 4 batches x 2 head-groups (8 heads each). Host pre-
transposes activations to [D, S], pre-slices/transposes weights, and casts
everything (except biases) to bf16 — fp32 matmuls stream at ~1.8 cyc/col on
TRN2 while bf16 streams at 1 cyc/col, and the rel-err budget (2e-2) has
~4x headroom over the measured all-bf16 error (~6e-3).

Pipeline (per s-chunk of 512, per head-pair p):
  - t-loop over 16 key tiles: two K=64 scores matmuls (row-tiled at
    partitions 0/64, run concurrently on the PE) write a 2-bank PSUM tile;
    ScalarE exps it (scale=1/8 folded, no max-subtraction) into bf16 SBUF;
    the PV matmuls for t-1 (65-row V_aug with an appended ones column
    yielding output + softmax denominator) accumulate into 2 PSUM banks.
  - Scores PSUM is double-buffered (2+2 banks) so scores(t+1) overlaps
    exp(t): the ScalarE exp stream (~266us) is the critical engine; all
    other work (o-proj of the previous chunk, q-proj of the next chunk,
    softmax normalization) is sliced into small pieces and slotted into
    the t-loop so the PE rides under the ScalarE ceiling.
  - Normalization: DVE reciprocal_approx_fast on the denominator row
    (~5x faster than exact reciprocal), K=1 ones-matmul broadcast across
    partitions, one DVE multiply into bf16 attn tiles.
  - O-projection contracts attn^T [d_local, s] against w_o columns;
    per-core partial outputs are summed (+b_o) on the host.
"""
import numpy as np
from ml_dtypes import bfloat16

import concourse.bass as bass
import concourse.mybir as mybir
from concourse.tile import TileContext
from concourse.bass_utils import run_bass_kernel_spmd

F32 = mybir.dt.float32
BF16 = mybir.dt.bfloat16
AF = mybir.ActivationFunctionType

B, S, D = 4, 2048, 1024
H, DH = 16, 64
HL = 8        # heads per core
DL = HL * DH  # 512 local model dims
P = 128
SC = 512      # s-chunk width
NCH = S // SC  # 4 s-chunks
ND = D // P    # 8 contraction subtiles for D
NM = DL // P   # 4 m-tiles of local outputs
NT = S // P    # 16 t-tiles

_MAX_WAITS = 1


def _split_excess_waits(nc, max_waits=_MAX_WAITS):
    """walrus here rejects >1 sync-wait per instruction; spill extras onto
    same-engine NoOps inserted before the instruction."""
    f = nc.m.functions[0]
    n = 0
    for bb in f.blocks:
        changed = False
        out = []
        for inst in bb.instructions:
            si = inst.sync_info
            if si is not None and len(si.on_wait) > max_waits:
                waits = list(si.on_wait)
                keep = waits[-max_waits:]
                spill = waits[:-max_waits]
                for i in range(0, len(spill), max_waits):
                    nop = mybir.InstNoOp(name=f"WSPILL-{n}", ins=[], outs=[])
                    n += 1
                    nop.engine = inst.engine
                    nop.sync_info = mybir.SyncInfo(
                        on_wait=spill[i : i + max_waits], on_update=[]
                    )
                    nc.register_instruction(nop, overwrite=True)
                    out.append(nop)
                inst.sync_info = mybir.SyncInfo(
                    on_wait=keep, on_update=list(si.on_update)
                )
                changed = True
            out.append(inst)
        if changed:
            bb.instructions = out
    return n


def build():
    nc = bass.Bass()
    xq = nc.dram_tensor("xq", [D, S], BF16, kind="ExternalInput")
    xk = nc.dram_tensor("xk", [D, S], BF16, kind="ExternalInput")
    xv = nc.dram_tensor("xv", [D, S], BF16, kind="ExternalInput")
    wq = nc.dram_tensor("wq", [D, DL], BF16, kind="ExternalInput")
    wk = nc.dram_tensor("wk", [D, DL], BF16, kind="ExternalInput")
    wv = nc.dram_tensor("wv", [D, DL], BF16, kind="ExternalInput")
    wo = nc.dram_tensor("wo", [DL, D], BF16, kind="ExternalInput")
    bq = nc.dram_tensor("bq", [DL], F32, kind="ExternalInput")
    bk = nc.dram_tensor("bk", [DL], F32, kind="ExternalInput")
    bv = nc.dram_tensor("bv", [DL], F32, kind="ExternalInput")
    out = nc.dram_tensor("out", [S, D], F32, kind="ExternalOutput")

    xq_r = xq.rearrange("(o p) s -> p o s", p=P)
    xk_r = xk.rearrange("(o p) s -> p o s", p=P)
    xv_r = xv.rearrange("(o p) s -> p o s", p=P)
    wq_r = wq.rearrange("(o p) m -> p o m", p=P)
    wk_r = wk.rearrange("(o p) m -> p o m", p=P)
    wv_r = wv.rearrange("(o p) m -> p o m", p=P)
    wo_r = wo.rearrange("(o p) n -> p o n", p=P)
    out_r = out.rearrange("(so p) n -> p so n", p=P)

    with TileContext(nc) as tc:
        with (
            tc.tile_pool(name="persist", bufs=1) as persist,
            tc.tile_pool(name="wpool", bufs=1) as wpool,
            tc.tile_pool(name="xpool", bufs=3) as xpool,
            tc.tile_pool(name="expp", bufs=4) as expp,
            tc.tile_pool(name="attnp", bufs=2) as attnp,
            tc.tile_pool(name="osb", bufs=2) as osbp,
            tc.tile_pool(name="nrm", bufs=2) as nrm,
            tc.tile_pool(name="ps_sc", bufs=2, space="PSUM") as ps_sc,
            tc.tile_pool(name="ps_pv", bufs=1, space="PSUM") as ps_pv,
            tc.tile_pool(name="ps_sm", bufs=2, space="PSUM") as ps_sm,
        ):
            qt = persist.tile([P, NM, S], BF16, tag="qt")
            kt = persist.tile([P, NM, S], BF16, tag="kt")
            vaug = persist.tile([P, NT, HL, 66], BF16, tag="vaug")
            wq_t = persist.tile([P, ND, DL], BF16, tag="wq")
            ones_f = persist.tile([P, P], F32, tag="ones_f")
            bq_sb = persist.tile([P, NM], F32, tag="bq")
            bk_sb = persist.tile([P, NM], F32, tag="bk")
            bv_t = persist.tile([P, DL], F32, tag="bv_t")
            bv_bc = persist.tile([P, DL], F32, tag="bv_bc")

            # ---- constants / biases ----
            nc.vector.memset(ones_f[:], 1.0)
            nc.sync.dma_start(bq_sb[:], bq.rearrange("(o p) -> p o", p=P))
            nc.sync.dma_start(bk_sb[:], bk.rearrange("(o p) -> p o", p=P))
            nc.sync.dma_start(bv_t[0:1, :], bv[None, :])
            ps = ps_sm.tile([P, SC], F32, tag="sm")
            nc.tensor.matmul(ps[:], ones_f[0:1, 0:P], bv_t[0:1, :], start=True,
                             stop=True)
            nc.vector.tensor_copy(bv_bc[:], ps[:])
            # ones column of V_aug (output row 64 = softmax denominator)
            of = ones_f[:, 0:NT * HL].rearrange("p (a b) -> p a b", a=NT)
            nc.vector.tensor_copy(vaug[:, :, :, 65:66], of[:, :, :, None])

            def proj_dma(x_r, c):
                xa = xpool.tile([P, ND // 2, SC], BF16, tag="x")
                xb = xpool.tile([P, ND // 2, SC], BF16, tag="x")
                nc.sync.dma_start(
                    xa[:], x_r[:, 0 : ND // 2, c * SC : (c + 1) * SC])
                nc.sync.dma_start(
                    xb[:], x_r[:, ND // 2 : ND, c * SC : (c + 1) * SC])
                return (xa, xb)

            def proj_chunk(dst, w_tile, x_r, c, bias_sb, m_tiles=range(NM),
                           halves=None):
                """Project one 512-col s-chunk for the given m-tiles."""
                if halves is None:
                    halves = proj_dma(x_r, c)
                for m in m_tiles:
                    psm = ps_sm.tile([P, SC], F32, tag="sm")
                    for k in range(ND):
                        nc.tensor.matmul(
                            psm[:],
                            w_tile[:, k, m * P : (m + 1) * P],
                            halves[k // 4][:, k % 4, :],
                            start=(k == 0),
                            stop=(k == ND - 1),
                        )
                    nc.vector.tensor_add(
                        dst[:, m, c * SC : (c + 1) * SC],
                        psm[:],
                        bias_sb[:, m : m + 1].to_broadcast((P, SC)),
                    )

            def vproj_chunk(wv_t, c):
                halves = proj_dma(xv_r, c)
                for i in range(4):
                    t_o = c * 4 + i
                    psm = ps_sm.tile([P, SC], F32, tag="sm")
                    for k in range(ND):
                        nc.tensor.matmul(
                            psm[:],
                            halves[k // 4][:, k % 4, i * P : (i + 1) * P],
                            wv_t[:, k, :],
                            start=(k == 0),
                            stop=(k == ND - 1),
                        )
                    nc.vector.tensor_add(
                        vaug[:, t_o, :, 1:65],
                        psm[:].rearrange("p (h d) -> p h d", h=HL),
                        bv_bc[:].rearrange("p (h d) -> p h d", h=HL),
                    )

            # ---- projections (prefix) ----
            nc.sync.dma_start(wq_t[:], wq_r[:])
            proj_chunk(qt, wq_t, xq_r, 0, bq_sb)
            wk_t = wpool.tile([P, ND, DL], BF16, tag="w")
            nc.sync.dma_start(wk_t[:], wk_r[:])
            for c in range(NCH):
                proj_chunk(kt, wk_t, xk_r, c, bk_sb)
            wv_t = wpool.tile([P, ND, DL], BF16, tag="w")
            nc.sync.dma_start(wv_t[:], wv_r[:])
            for c in range(NCH):
                vproj_chunk(wv_t, c)
            wo_t = wpool.tile([P, NM, D], BF16, tag="w")
            nc.sync.dma_start(wo_t[:], wo_r[:])

            # ---- attention + o-proj, per s-chunk ----
            def oproj_piece(c, attn_t, st, n):
                """One [128 s x 512 n] output tile of the o-projection."""
                psm = ps_sm.tile([P, SC], F32, tag="sm")
                for do in range(NM):
                    nc.tensor.matmul(
                        psm[:],
                        attn_t[:, do, st * P : (st + 1) * P],
                        wo_t[:, do, n * SC : (n + 1) * SC],
                        start=(do == 0),
                        stop=(do == NM - 1),
                    )
                ob = osbp.tile([P, SC], F32, tag="ob")
                nc.vector.tensor_copy(ob[:], psm[:])
                nc.sync.dma_start(
                    out_r[:, c * 4 + st, n * SC : (n + 1) * SC], ob[:]
                )

            def normalize_pre(pv):
                """Consume the PV psum right away on DVE (frees the psum slot):
                copy unnormalized rows, fast-reciprocal of the denominator."""
                raw = nrm.tile([P, SC], F32, tag="raw")
                rec = nrm.tile([P, SC], F32, tag="rec")
                nc.vector.tensor_copy(raw[0:65, :], pv[0:65, :])
                nc.vector.reciprocal(rec[64:65, :], raw[64:65, :])
                return raw, rec

            def normalize_post(raw, rec, dst):
                """Broadcast 1/denom across partitions (K=1 matmul), apply."""
                bc = ps_sm.tile([P, SC], F32, tag="sm")
                nc.tensor.matmul(bc[0:64, :], ones_f[64:65, 0:64], rec[64:65, :],
                                 start=True, stop=True)
                nc.vector.tensor_mul(dst, bc[0:64, :], raw[0:64, :])

            pending_norm = None   # (raw_e, rec_e, raw_o, rec_o, p, attn_t)
            pending_oproj = None  # (c, attn_t)
            for c in range(NCH):
                attn_t = attnp.tile([P, NM, SC], BF16, tag="attn")
                cs = slice(c * SC, (c + 1) * SC)
                for p in range(NM):
                    pv_e = ps_pv.tile([P, SC], F32, tag="pv_e")
                    pv_o = ps_pv.tile([P, SC], F32, tag="pv_o")
                    pend = None

                    def emit_pv(ex, t, p=p, pv_e=pv_e, pv_o=pv_o):
                        for j, (pv, hh) in enumerate(
                            [(pv_e, 2 * p), (pv_o, 2 * p + 1)]
                        ):
                            nc.tensor.matmul(
                                pv[0:65, :],
                                vaug[:, t, hh, 1:66],
                                ex[:, j, :],
                                start=(t == 0),
                                stop=(t == NT - 1),
                            )

                    for t in range(NT):
                        sct = ps_sc.tile([P, 2, SC], F32, tag="sc")
                        tw = slice(t * P, (t + 1) * P)
                        nc.tensor.matmul(
                            sct[:, 0, :], kt[0:64, p, tw], qt[0:64, p, cs],
                            start=True, stop=True, tile_position=(0, 0),
                        )
                        nc.tensor.matmul(
                            sct[:, 1, :], kt[64:128, p, tw], qt[64:128, p, cs],
                            start=True, stop=True, tile_position=(64, 0),
                        )
                        ex = expp.tile([P, 2, SC], BF16, tag="ex")
                        nc.scalar.activation(ex[:], sct[:], AF.Exp, scale=0.125)
                        if pend is not None:
                            emit_pv(*pend)
                        pend = (ex, t)
                        # -- interleaved off-ceiling work --
                        # (emitted after the PV pair so the scores of t+1
                        # stay close behind in the PE queue)
                        if t == 2 and pending_norm is not None:
                            raw_e, rec_e, raw_o, rec_o, pp, at = pending_norm
                            normalize_post(raw_e, rec_e, at[0:64, pp, :])
                            normalize_post(raw_o, rec_o, at[64:128, pp, :])
                            pending_norm = None
                        if p == 0 and pending_oproj is not None and \
                                t in (4, 6, 8, 10, 12, 14):
                            oc, oat = pending_oproj
                            i = (t - 4) // 2
                            oproj_piece(oc, oat, i // 2, i % 2)
                        if p == 1 and pending_oproj is not None and \
                                t in (2, 6):
                            oc, oat = pending_oproj
                            i = 6 + (t - 2) // 4
                            oproj_piece(oc, oat, i // 2, i % 2)
                            if i == 7:
                                pending_oproj = None
                        if p == 2 and c < NCH - 1 and t == 8:
                            next_halves = proj_dma(xq_r, c + 1)
                        if p == 3 and c < NCH - 1 and t in (2, 6, 10, 14):
                            proj_chunk(qt, wq_t, xq_r, c + 1, bq_sb,
                                       m_tiles=[(t - 2) // 4],
                                       halves=next_halves)
                    emit_pv(*pend)
                    raw_e, rec_e = normalize_pre(pv_e)
                    raw_o, rec_o = normalize_pre(pv_o)
                    pending_norm = (raw_e, rec_e, raw_o, rec_o, p, attn_t)

                # last head-pair of the chunk: normalize immediately so the
                # o-projection of this chunk can start early next chunk
                raw_e, rec_e, raw_o, rec_o, pp, at = pending_norm
                normalize_post(raw_e, rec_e, at[0:64, pp, :])
                normalize_post(raw_o, rec_o, at[64:128, pp, :])
                pending_norm = None
                pending_oproj = (c, attn_t)

            # drain: o-projection of the last chunk
            oc, oat = pending_oproj
            for st in range(4):
                for n in range(2):
                    oproj_piece(oc, oat, st, n)

    _split_excess_waits(nc)
    return nc


_CACHE = {}


def _get_nc():
    if "nc" not in _CACHE:
        _CACHE["nc"] = build()
    return _CACHE["nc"]


def _f32(x):
    return np.asarray(x).astype(np.float32, copy=False)


def _bf16(x):
    return np.ascontiguousarray(x.astype(bfloat16))


def _prep_core_inputs(c, q, k, v, w_q, b_q, w_k, b_k, w_v, b_v, w_o, b_o):
    b, hg = c // 2, c % 2
    hs = slice(hg * DL, hg * DL + DL)
    return {
        "xq": _bf16(q[b].T),
        "xk": _bf16(k[b].T),
        "xv": _bf16(v[b].T),
        "wq": _bf16(w_q[hs, :].T),
        "wk": _bf16(w_k[hs, :].T),
        "wv": _bf16(w_v[hs, :].T),
        "wo": _bf16(w_o[:, hs].T),
        "bq": np.ascontiguousarray(b_q[hs]),
        "bk": np.ascontiguousarray(b_k[hs]),
        "bv": np.ascontiguousarray(b_v[hs]),
    }


def kernel(q, k, v, w_q, b_q, w_k, b_k, w_v, b_v, w_o, b_o):
    q, k, v = _f32(q), _f32(k), _f32(v)
    w_q, b_q = _f32(w_q), _f32(b_q)
    w_k, b_k = _f32(w_k), _f32(b_k)
    w_v, b_v = _f32(w_v), _f32(b_v)
    w_o, b_o = _f32(w_o), _f32(b_o)

    nc = _get_nc()
    in_maps = [
        _prep_core_inputs(c, q, k, v, w_q, b_q, w_k, b_k, w_v, b_v, w_o, b_o)
        for c in range(8)
    ]
    res = run_bass_kernel_spmd(nc, in_maps, core_ids=list(range(8)))
    out = np.empty((B, S, D), np.float32)
    for b in range(B):
        out[b] = res.results[2 * b]["out"] + res.results[2 * b + 1]["out"] + b_o
    return out


# revision 11
# speedup vs baseline: 1.2464x; 1.1215x over previous
"""Trainium2 Bass SPMD kernel: 16-head MHA (B=4, S=2048, D=1024), fp32 in/out.

Sharding: 8 cores = 4 batches x 2 head-groups (8 heads each). Host pre-
transposes activations to [D, S], pre-slices/transposes weights, and casts
everything (except biases) to bf16 — fp32 matmuls stream at ~1.8 cyc/col on
TRN2 while bf16 streams at 1 cyc/col, and the rel-err budget (2e-2) has
~4x headroom over the measured all-bf16 error (~6e-3).

Pipeline (per s-chunk of 512, per head-pair p):
  - t-loop over 16 key tiles: two K=64 scores matmuls (row-tiled at
    partitions 0/64, run concurrently on the PE) write a 2-bank PSUM tile;
    ScalarE exps it (scale=1/8 folded, no max-subtraction) into bf16 SBUF;
    the PV matmuls for t-1 (65-row V_aug with an appended ones column
    yielding output + softmax denominator) accumulate into 2 PSUM banks.
  - Scores PSUM is double-buffered (2+2 banks) so scores(t+1) overlaps
    exp(t): the ScalarE exp stream (~266us) is the critical engine; all
    other work (o-proj of the previous chunk, q-proj of the next chunk,
    softmax normalization) is sliced into small pieces and slotted into
    the t-loop so the PE rides under the ScalarE ceiling.
  - Normalization: DVE reciprocal_approx_fast on the denominator row
    (~5x faster than exact reciprocal), K=1 ones-matmul broadcast across
    partitions, one DVE multiply into bf16 attn tiles.
  - O-projection contracts attn^T [d_local, s] against w_o columns;
    per-core partial outputs are summed (+b_o) on the host.
"""
import numpy as np
from ml_dtypes import bfloat16

import concourse.bass as bass
import concourse.mybir as mybir
from concourse.tile import TileContext
from concourse.bass_utils import run_bass_kernel_spmd

F32 = mybir.dt.float32
BF16 = mybir.dt.bfloat16
AF = mybir.ActivationFunctionType

B, S, D = 4, 2048, 1024
H, DH = 16, 64
HL = 8        # heads per core
DL = HL * DH  # 512 local model dims
P = 128
SC = 512      # s-chunk width
NCH = S // SC  # 4 s-chunks
ND = D // P    # 8 contraction subtiles for D
NM = DL // P   # 4 m-tiles of local outputs
NT = S // P    # 16 t-tiles

_MAX_WAITS = 1


def _split_excess_waits(nc, max_waits=_MAX_WAITS):
    """walrus here rejects >1 sync-wait per instruction; spill extras onto
    same-engine NoOps inserted before the instruction."""
    f = nc.m.functions[0]
    n = 0
    for bb in f.blocks:
        changed = False
        out = []
        for inst in bb.instructions:
            si = inst.sync_info
            if si is not None and len(si.on_wait) > max_waits:
                waits = list(si.on_wait)
                keep = waits[-max_waits:]
                spill = waits[:-max_waits]
                for i in range(0, len(spill), max_waits):
                    nop = mybir.InstNoOp(name=f"WSPILL-{n}", ins=[], outs=[])
                    n += 1
                    nop.engine = inst.engine
                    nop.sync_info = mybir.SyncInfo(
                        on_wait=spill[i : i + max_waits], on_update=[]
                    )
                    nc.register_instruction(nop, overwrite=True)
                    out.append(nop)
                inst.sync_info = mybir.SyncInfo(
                    on_wait=keep, on_update=list(si.on_update)
                )
                changed = True
            out.append(inst)
        if changed:
            bb.instructions = out
    return n


def build():
    nc = bass.Bass()
    xq = nc.dram_tensor("xq", [D, S], BF16, kind="ExternalInput")
    xk = nc.dram_tensor("xk", [D, S], BF16, kind="ExternalInput")
    xv = nc.dram_tensor("xv", [D, S], BF16, kind="ExternalInput")
    wq = nc.dram_tensor("wq", [D, DL], BF16, kind="ExternalInput")
    wk = nc.dram_tensor("wk", [D, DL], BF16, kind="ExternalInput")
    wv = nc.dram_tensor("wv", [D, DL], BF16, kind="ExternalInput")
    wo = nc.dram_tensor("wo", [DL, D], BF16, kind="ExternalInput")
    bq = nc.dram_tensor("bq", [DL], F32, kind="ExternalInput")
    bk = nc.dram_tensor("bk", [DL], F32, kind="ExternalInput")
    bv = nc.dram_tensor("bv", [DL], F32, kind="ExternalInput")
    out = nc.dram_tensor("out", [S, D], F32, kind="ExternalOutput")

    xq_r = xq.rearrange("(o p) s -> p o s", p=P)
    xk_r = xk.rearrange("(o p) s -> p o s", p=P)
    xv_r = xv.rearrange("(o p) s -> p o s", p=P)
    wq_r = wq.rearrange("(o p) m -> p o m", p=P)
    wk_r = wk.rearrange("(o p) m -> p o m", p=P)
    wv_r = wv.rearrange("(o p) m -> p o m", p=P)
    wo_r = wo.rearrange("(o p) n -> p o n", p=P)
    out_r = out.rearrange("(so p) n -> p so n", p=P)

    with TileContext(nc) as tc:
        with (
            tc.tile_pool(name="persist", bufs=1) as persist,
            tc.tile_pool(name="wpool", bufs=1) as wpool,
            tc.tile_pool(name="xpool", bufs=12) as xpool,
            tc.tile_pool(name="expp", bufs=4) as expp,
            tc.tile_pool(name="attnp", bufs=2) as attnp,
            tc.tile_pool(name="osb", bufs=2) as osbp,
            tc.tile_pool(name="nrm", bufs=2) as nrm,
            tc.tile_pool(name="ps_sc", bufs=2, space="PSUM") as ps_sc,
            tc.tile_pool(name="ps_pv", bufs=1, space="PSUM") as ps_pv,
            tc.tile_pool(name="ps_sm", bufs=2, space="PSUM") as ps_sm,
        ):
            qt = persist.tile([P, NM, S], BF16, tag="qt")
            kt = persist.tile([P, NM, S], BF16, tag="kt")
            vaug = persist.tile([P, NT, HL, 66], BF16, tag="vaug")
            wq_t = persist.tile([P, ND, DL], BF16, tag="wq")
            ones_f = persist.tile([P, P], F32, tag="ones_f")
            bq_sb = persist.tile([P, NM], F32, tag="bq")
            bk_sb = persist.tile([P, NM], F32, tag="bk")
            bv_t = persist.tile([P, DL], F32, tag="bv_t")
            bv_bc = persist.tile([P, DL], F32, tag="bv_bc")

            # ---- constants / biases ----
            nc.vector.memset(ones_f[:], 1.0)
            nc.sync.dma_start(bq_sb[:], bq.rearrange("(o p) -> p o", p=P))
            nc.sync.dma_start(bk_sb[:], bk.rearrange("(o p) -> p o", p=P))
            nc.sync.dma_start(bv_t[0:1, :], bv[None, :])
            ps = ps_sm.tile([P, SC], F32, tag="sm")
            nc.tensor.matmul(ps[:], ones_f[0:1, 0:P], bv_t[0:1, :], start=True,
                             stop=True)
            nc.vector.tensor_copy(bv_bc[:], ps[:])
            # ones column of V_aug (output row 64 = softmax denominator)
            of = ones_f[:, 0:NT * HL].rearrange("p (a b) -> p a b", a=NT)
            nc.vector.tensor_copy(vaug[:, :, :, 65:66], of[:, :, :, None])

            def proj_dma(x_r, c):
                xa = xpool.tile([P, ND // 2, SC], BF16, tag="x")
                xb = xpool.tile([P, ND // 2, SC], BF16, tag="x")
                nc.sync.dma_start(
                    xa[:], x_r[:, 0 : ND // 2, c * SC : (c + 1) * SC])
                nc.sync.dma_start(
                    xb[:], x_r[:, ND // 2 : ND, c * SC : (c + 1) * SC])
                return (xa, xb)

            def proj_chunk(dst, w_tile, x_r, c, bias_sb, m_tiles=range(NM),
                           halves=None):
                """Project one 512-col s-chunk for the given m-tiles."""
                if halves is None:
                    halves = proj_dma(x_r, c)
                for m in m_tiles:
                    psm = ps_sm.tile([P, SC], F32, tag="sm")
                    for k in range(ND):
                        nc.tensor.matmul(
                            psm[:],
                            w_tile[:, k, m * P : (m + 1) * P],
                            halves[k // 4][:, k % 4, :],
                            start=(k == 0),
                            stop=(k == ND - 1),
                        )
                    nc.vector.tensor_add(
                        dst[:, m, c * SC : (c + 1) * SC],
                        psm[:],
                        bias_sb[:, m : m + 1].to_broadcast((P, SC)),
                    )

            def vproj_piece(wv_t, c, i, halves):
                t_o = c * 4 + i
                psm = ps_sm.tile([P, SC], F32, tag="sm")
                for k in range(ND):
                    nc.tensor.matmul(
                        psm[:],
                        halves[k // 4][:, k % 4, i * P : (i + 1) * P],
                        wv_t[:, k, :],
                        start=(k == 0),
                        stop=(k == ND - 1),
                    )
                nc.vector.tensor_add(
                    vaug[:, t_o, :, 1:65],
                    psm[:].rearrange("p (h d) -> p h d", h=HL),
                    bv_bc[:].rearrange("p (h d) -> p h d", h=HL),
                )

            # ---- prefix: chunk-0 projections; K/V chunks 1-3 are folded
            # into the first head-pair's t-loop to keep ScalarE fed early ----
            wk_t = wpool.tile([P, ND, DL], BF16, tag="wk")
            wv_t = wpool.tile([P, ND, DL], BF16, tag="wv")
            wo_t = wpool.tile([P, NM, D], BF16, tag="wo")
            nc.sync.dma_start(wq_t[:], wq_r[:])
            hq0 = proj_dma(xq_r, 0)
            nc.sync.dma_start(wk_t[:], wk_r[:])
            hk = {0: proj_dma(xk_r, 0)}
            nc.sync.dma_start(wv_t[:], wv_r[:])
            hv = {0: proj_dma(xv_r, 0)}
            nc.sync.dma_start(wo_t[:], wo_r[:])
            proj_chunk(qt, wq_t, xq_r, 0, bq_sb, halves=hq0)
            proj_chunk(kt, wk_t, xk_r, 0, bk_sb, halves=hk[0])
            hk[1] = proj_dma(xk_r, 1)
            for i in range(4):
                vproj_piece(wv_t, 0, i, hv[0])
            hv[1] = proj_dma(xv_r, 1)

            # ---- attention + o-proj, per s-chunk ----
            def oproj_piece(c, attn_t, st, n):
                """One [128 s x 512 n] output tile of the o-projection."""
                psm = ps_sm.tile([P, SC], F32, tag="sm")
                for do in range(NM):
                    nc.tensor.matmul(
                        psm[:],
                        attn_t[:, do, st * P : (st + 1) * P],
                        wo_t[:, do, n * SC : (n + 1) * SC],
                        start=(do == 0),
                        stop=(do == NM - 1),
                    )
                ob = osbp.tile([P, SC], F32, tag="ob")
                nc.vector.tensor_copy(ob[:], psm[:])
                nc.sync.dma_start(
                    out_r[:, c * 4 + st, n * SC : (n + 1) * SC], ob[:]
                )

            def normalize_pre(pv):
                """Consume the PV psum right away on DVE (frees the psum slot):
                copy unnormalized rows, fast-reciprocal of the denominator."""
                raw = nrm.tile([P, SC], F32, tag="raw")
                rec = nrm.tile([P, SC], F32, tag="rec")
                nc.vector.tensor_copy(raw[0:65, :], pv[0:65, :])
                nc.vector.reciprocal(rec[64:65, :], raw[64:65, :])
                return raw, rec

            def normalize_post(raw, rec, dst):
                """Broadcast 1/denom across partitions (K=1 matmul), apply."""
                bc = ps_sm.tile([P, SC], F32, tag="sm")
                nc.tensor.matmul(bc[0:64, :], ones_f[64:65, 0:64], rec[64:65, :],
                                 start=True, stop=True)
                nc.vector.tensor_mul(dst, bc[0:64, :], raw[0:64, :])

            pending_norm = None   # (raw_e, rec_e, raw_o, rec_o, p, attn_t)
            pending_oproj = None  # (c, attn_t)
            for c in range(NCH):
                attn_t = attnp.tile([P, NM, SC], BF16, tag="attn")
                cs = slice(c * SC, (c + 1) * SC)
                for p in range(NM):
                    pv_e = ps_pv.tile([P, SC], F32, tag="pv_e")
                    pv_o = ps_pv.tile([P, SC], F32, tag="pv_o")
                    pend = None

                    def emit_pv(ex, t, p=p, pv_e=pv_e, pv_o=pv_o):
                        for j, (pv, hh) in enumerate(
                            [(pv_e, 2 * p), (pv_o, 2 * p + 1)]
                        ):
                            nc.tensor.matmul(
                                pv[0:65, :],
                                vaug[:, t, hh, 1:66],
                                ex[:, j, :],
                                start=(t == 0),
                                stop=(t == NT - 1),
                            )

                    for t in range(NT):
                        sct = ps_sc.tile([P, 2, SC], F32, tag="sc")
                        tw = slice(t * P, (t + 1) * P)
                        nc.tensor.matmul(
                            sct[:, 0, :], kt[0:64, p, tw], qt[0:64, p, cs],
                            start=True, stop=True, tile_position=(0, 0),
                        )
                        nc.tensor.matmul(
                            sct[:, 1, :], kt[64:128, p, tw], qt[64:128, p, cs],
                            start=True, stop=True, tile_position=(64, 0),
                        )
                        ex = expp.tile([P, 2, SC], BF16, tag="ex")
                        nc.scalar.activation(ex[:], sct[:], AF.Exp, scale=0.125)
                        if pend is not None:
                            emit_pv(*pend)
                        pend = (ex, t)
                        # -- interleaved off-ceiling work (after the PV pair
                        # so a piece stall can't starve the exp stream) --
                        if pending_norm is not None:
                            ns = 0 if p == 0 else 5
                            if t == ns:
                                raw_e, rec_e, _, _, pp, at = pending_norm
                                normalize_post(raw_e, rec_e, at[0:64, pp, :])
                            elif t == ns + 2:
                                _, _, raw_o, rec_o, pp, at = pending_norm
                                normalize_post(raw_o, rec_o, at[64:128, pp, :])
                                pending_norm = None
                        if c == 0 and p == 0 and t < 12:
                            g = t // 4 + 1
                            if t == 0:
                                hk[2] = proj_dma(xk_r, 2)
                            elif t == 2:
                                hv[2] = proj_dma(xv_r, 2)
                            elif t == 4:
                                hk[3] = proj_dma(xk_r, 3)
                            elif t == 6:
                                hv[3] = proj_dma(xv_r, 3)
                            proj_chunk(kt, wk_t, xk_r, g, bk_sb,
                                       m_tiles=[t % 4], halves=hk[g])
                            vproj_piece(wv_t, g, t % 4, hv[g])
                        if p == 0 and pending_oproj is not None and \
                                t in (6, 8, 10, 12, 14):
                            oc, oat = pending_oproj
                            i = (t - 6) // 2
                            oproj_piece(oc, oat, i // 2, i % 2)
                        if p == 1 and pending_oproj is not None and \
                                t in (2, 4, 6):
                            oc, oat = pending_oproj
                            i = 5 + (t - 2) // 2
                            oproj_piece(oc, oat, i // 2, i % 2)
                            if i == 7:
                                pending_oproj = None
                        if p == 2 and c < NCH - 1 and t == 8:
                            next_halves = proj_dma(xq_r, c + 1)
                        if p == 3 and c < NCH - 1 and t in (2, 6, 10, 14):
                            proj_chunk(qt, wq_t, xq_r, c + 1, bq_sb,
                                       m_tiles=[(t - 2) // 4],
                                       halves=next_halves)
                    emit_pv(*pend)
                    raw_e, rec_e = normalize_pre(pv_e)
                    raw_o, rec_o = normalize_pre(pv_o)
                    pending_norm = (raw_e, rec_e, raw_o, rec_o, p, attn_t)

                pending_oproj = (c, attn_t)

            # drain: normalize of the last head-pair + o-projection of the
            # last chunk
            raw_e, rec_e, raw_o, rec_o, pp, at = pending_norm
            normalize_post(raw_e, rec_e, at[0:64, pp, :])
            normalize_post(raw_o, rec_o, at[64:128, pp, :])
            oc, oat = pending_oproj
            for st in range(4):
                for n in range(2):
                    oproj_piece(oc, oat, st, n)

    _split_excess_waits(nc)
    return nc


_CACHE = {}


def _get_nc():
    if "nc" not in _CACHE:
        _CACHE["nc"] = build()
    return _CACHE["nc"]


def _f32(x):
    return np.asarray(x).astype(np.float32, copy=False)


def _bf16(x):
    return np.ascontiguousarray(x.astype(bfloat16))


def _prep_core_inputs(c, q, k, v, w_q, b_q, w_k, b_k, w_v, b_v, w_o, b_o):
    b, hg = c // 2, c % 2
    hs = slice(hg * DL, hg * DL + DL)
    return {
        "xq": _bf16(q[b].T),
        "xk": _bf16(k[b].T),
        "xv": _bf16(v[b].T),
        "wq": _bf16(w_q[hs, :].T),
        "wk": _bf16(w_k[hs, :].T),
        "wv": _bf16(w_v[hs, :].T),
        "wo": _bf16(w_o[:, hs].T),
        "bq": np.ascontiguousarray(b_q[hs]),
        "bk": np.ascontiguousarray(b_k[hs]),
        "bv": np.ascontiguousarray(b_v[hs]),
    }


def kernel(q, k, v, w_q, b_q, w_k, b_k, w_v, b_v, w_o, b_o):
    q, k, v = _f32(q), _f32(k), _f32(v)
    w_q, b_q = _f32(w_q), _f32(b_q)
    w_k, b_k = _f32(w_k), _f32(b_k)
    w_v, b_v = _f32(w_v), _f32(b_v)
    w_o, b_o = _f32(w_o), _f32(b_o)

    nc = _get_nc()
    in_maps = [
        _prep_core_inputs(c, q, k, v, w_q, b_q, w_k, b_k, w_v, b_v, w_o, b_o)
        for c in range(8)
    ]
    res = run_bass_kernel_spmd(nc, in_maps, core_ids=list(range(8)))
    out = np.empty((B, S, D), np.float32)
    for b in range(B):
        out[b] = res.results[2 * b]["out"] + res.results[2 * b + 1]["out"] + b_o
    return out


# revision 12
# speedup vs baseline: 1.3731x; 1.1017x over previous
"""Trainium2 Bass SPMD kernel: 16-head MHA (B=4, S=2048, D=1024), fp32 in/out.

Sharding: 8 cores = 4 batches x 2 head-groups (8 heads each). Host pre-
transposes activations to [D, S], pre-slices/transposes weights, and casts
everything (except biases) to bf16 — fp32 matmuls stream at ~1.8 cyc/col on
TRN2 while bf16 streams at 1 cyc/col, and the rel-err budget (2e-2) has
~4x headroom over the measured all-bf16 error (~6e-3).

Pipeline (per s-chunk of 512, per head-pair p):
  - t-loop over 16 key tiles: two K=64 scores matmuls (row-tiled at
    partitions 0/64, run concurrently on the PE) write a 2-bank PSUM tile;
    ScalarE exps it (scale=1/8 folded, no max-subtraction) into bf16 SBUF;
    the PV matmuls for t-1 (65-row V_aug with an appended ones column
    yielding output + softmax denominator) accumulate into 2 PSUM banks.
  - Scores PSUM is double-buffered (2+2 banks) so scores(t+1) overlaps
    exp(t): the ScalarE exp stream (~266us) is the critical engine; all
    other work (o-proj of the previous chunk, q-proj of the next chunk,
    softmax normalization) is sliced into small pieces and slotted into
    the t-loop so the PE rides under the ScalarE ceiling.
  - Normalization: DVE reciprocal_approx_fast on the denominator row
    (~5x faster than exact reciprocal), K=1 ones-matmul broadcast across
    partitions, one DVE multiply into bf16 attn tiles.
  - O-projection contracts attn^T [d_local, s] against w_o columns;
    per-core partial outputs are summed (+b_o) on the host.
"""
import numpy as np
from ml_dtypes import bfloat16

import concourse.bass as bass
import concourse.mybir as mybir
from concourse.tile import TileContext
from concourse.bass_utils import run_bass_kernel_spmd

F32 = mybir.dt.float32
BF16 = mybir.dt.bfloat16
AF = mybir.ActivationFunctionType

B, S, D = 4, 2048, 1024
H, DH = 16, 64
HL = 8        # heads per core
DL = HL * DH  # 512 local model dims
P = 128
SC = 512      # s-chunk width
NCH = S // SC  # 4 s-chunks
ND = D // P    # 8 contraction subtiles for D
NM = DL // P   # 4 m-tiles of local outputs
NT = S // P    # 16 t-tiles

_MAX_WAITS = 1


def _split_excess_waits(nc, max_waits=_MAX_WAITS):
    """walrus here rejects >1 sync-wait per instruction; spill extras onto
    same-engine NoOps inserted before the instruction."""
    f = nc.m.functions[0]
    n = 0
    for bb in f.blocks:
        changed = False
        out = []
        for inst in bb.instructions:
            si = inst.sync_info
            if si is not None and len(si.on_wait) > max_waits:
                waits = list(si.on_wait)
                keep = waits[-max_waits:]
                spill = waits[:-max_waits]
                for i in range(0, len(spill), max_waits):
                    nop = mybir.InstNoOp(name=f"WSPILL-{n}", ins=[], outs=[])
                    n += 1
                    nop.engine = inst.engine
                    nop.sync_info = mybir.SyncInfo(
                        on_wait=spill[i : i + max_waits], on_update=[]
                    )
                    nc.register_instruction(nop, overwrite=True)
                    out.append(nop)
                inst.sync_info = mybir.SyncInfo(
                    on_wait=keep, on_update=list(si.on_update)
                )
                changed = True
            out.append(inst)
        if changed:
            bb.instructions = out
    return n


def build():
    nc = bass.Bass()
    xq = nc.dram_tensor("xq", [D, S], BF16, kind="ExternalInput")
    xk = nc.dram_tensor("xk", [D, S], BF16, kind="ExternalInput")
    xv = nc.dram_tensor("xv", [D, S], BF16, kind="ExternalInput")
    wq = nc.dram_tensor("wq", [D, DL], BF16, kind="ExternalInput")
    wk = nc.dram_tensor("wk", [D, DL], BF16, kind="ExternalInput")
    wv = nc.dram_tensor("wv", [D, DL], BF16, kind="ExternalInput")
    wo = nc.dram_tensor("wo", [DL, D], BF16, kind="ExternalInput")
    bq = nc.dram_tensor("bq", [DL], F32, kind="ExternalInput")
    bk = nc.dram_tensor("bk", [DL], F32, kind="ExternalInput")
    bv = nc.dram_tensor("bv", [DL], F32, kind="ExternalInput")
    out = nc.dram_tensor("out", [S, D], F32, kind="ExternalOutput")

    xq_r = xq.rearrange("(o p) s -> p o s", p=P)
    xk_r = xk.rearrange("(o p) s -> p o s", p=P)
    xv_r = xv.rearrange("(o p) s -> p o s", p=P)
    wq_r = wq.rearrange("(o p) m -> p o m", p=P)
    wk_r = wk.rearrange("(o p) m -> p o m", p=P)
    wv_r = wv.rearrange("(o p) m -> p o m", p=P)
    wo_r = wo.rearrange("(o p) n -> p o n", p=P)
    out_r = out.rearrange("(so p) n -> p so n", p=P)

    with TileContext(nc) as tc:
        with (
            tc.tile_pool(name="persist", bufs=1) as persist,
            tc.tile_pool(name="wpool", bufs=1) as wpool,
            tc.tile_pool(name="xpool", bufs=12) as xpool,
            tc.tile_pool(name="expp", bufs=4) as expp,
            tc.tile_pool(name="attnp", bufs=2) as attnp,
            tc.tile_pool(name="osb", bufs=2) as osbp,
            tc.tile_pool(name="nrm", bufs=2) as nrm,
            tc.tile_pool(name="ps_sc", bufs=2, space="PSUM") as ps_sc,
            tc.tile_pool(name="ps_pv", bufs=1, space="PSUM") as ps_pv,
            tc.tile_pool(name="ps_sm", bufs=2, space="PSUM") as ps_sm,
        ):
            qt = persist.tile([P, NM, S], BF16, tag="qt")
            kt = persist.tile([P, NM, S], BF16, tag="kt")
            vaug = persist.tile([P, NT, HL, 66], BF16, tag="vaug")
            wq_t = persist.tile([P, ND, DL], BF16, tag="wq")
            ones_f = persist.tile([P, P], F32, tag="ones_f")
            bq_sb = persist.tile([P, NM], F32, tag="bq")
            bk_sb = persist.tile([P, NM], F32, tag="bk")
            bv_t = persist.tile([P, DL], F32, tag="bv_t")
            bv_bc = persist.tile([P, DL], F32, tag="bv_bc")

            # ---- constants / biases ----
            nc.vector.memset(ones_f[:], 1.0)
            nc.sync.dma_start(bq_sb[:], bq.rearrange("(o p) -> p o", p=P))
            nc.sync.dma_start(bk_sb[:], bk.rearrange("(o p) -> p o", p=P))
            nc.sync.dma_start(bv_t[0:1, :], bv[None, :])
            ps = ps_sm.tile([P, SC], F32, tag="sm")
            nc.tensor.matmul(ps[:], ones_f[0:1, 0:P], bv_t[0:1, :], start=True,
                             stop=True)
            nc.vector.tensor_copy(bv_bc[:], ps[:])
            # ones column of V_aug (output row 64 = softmax denominator)
            of = ones_f[:, 0:NT * HL].rearrange("p (a b) -> p a b", a=NT)
            nc.vector.tensor_copy(vaug[:, :, :, 65:66], of[:, :, :, None])

            def proj_dma(x_r, c):
                xa = xpool.tile([P, ND // 2, SC], BF16, tag="x")
                xb = xpool.tile([P, ND // 2, SC], BF16, tag="x")
                nc.sync.dma_start(
                    xa[:], x_r[:, 0 : ND // 2, c * SC : (c + 1) * SC])
                nc.sync.dma_start(
                    xb[:], x_r[:, ND // 2 : ND, c * SC : (c + 1) * SC])
                return (xa, xb)

            def proj_chunk(dst, w_tile, x_r, c, bias_sb, m_tiles=range(NM),
                           halves=None):
                """Project one 512-col s-chunk for the given m-tiles."""
                if halves is None:
                    halves = proj_dma(x_r, c)
                for m in m_tiles:
                    psm = ps_sm.tile([P, SC], F32, tag="sm")
                    for k in range(ND):
                        nc.tensor.matmul(
                            psm[:],
                            w_tile[:, k, m * P : (m + 1) * P],
                            halves[k // 4][:, k % 4, :],
                            start=(k == 0),
                            stop=(k == ND - 1),
                        )
                    nc.vector.tensor_add(
                        dst[:, m, c * SC : (c + 1) * SC],
                        psm[:],
                        bias_sb[:, m : m + 1].to_broadcast((P, SC)),
                    )

            def vproj_piece(wv_t, c, i, halves):
                t_o = c * 4 + i
                psm = ps_sm.tile([P, SC], F32, tag="sm")
                for k in range(ND):
                    nc.tensor.matmul(
                        psm[:],
                        halves[k // 4][:, k % 4, i * P : (i + 1) * P],
                        wv_t[:, k, :],
                        start=(k == 0),
                        stop=(k == ND - 1),
                    )
                nc.vector.tensor_add(
                    vaug[:, t_o, :, 1:65],
                    psm[:].rearrange("p (h d) -> p h d", h=HL),
                    bv_bc[:].rearrange("p (h d) -> p h d", h=HL),
                )

            # ---- prefix: chunk-0 projections; K/V chunks 1-3 are folded
            # into the first head-pair's t-loop to keep ScalarE fed early ----
            wk_t = wpool.tile([P, ND, DL], BF16, tag="wk")
            wv_t = wpool.tile([P, ND, DL], BF16, tag="wv")
            wo_t = wpool.tile([P, NM, D], BF16, tag="wo")
            nc.sync.dma_start(wq_t[:], wq_r[:])
            hq0 = proj_dma(xq_r, 0)
            nc.sync.dma_start(wk_t[:], wk_r[:])
            hk = {0: proj_dma(xk_r, 0)}
            nc.sync.dma_start(wv_t[:], wv_r[:])
            hv = {0: proj_dma(xv_r, 0)}
            nc.sync.dma_start(wo_t[:], wo_r[:])
            proj_chunk(qt, wq_t, xq_r, 0, bq_sb, halves=hq0)
            proj_chunk(kt, wk_t, xk_r, 0, bk_sb, halves=hk[0])
            hk[1] = proj_dma(xk_r, 1)
            for i in range(4):
                vproj_piece(wv_t, 0, i, hv[0])
            hv[1] = proj_dma(xv_r, 1)

            # ---- attention + o-proj, per s-chunk ----
            def oproj_piece(c, attn_t, st, n):
                """One [128 s x 512 n] output tile of the o-projection."""
                psm = ps_sm.tile([P, SC], F32, tag="sm")
                for do in range(NM):
                    nc.tensor.matmul(
                        psm[:],
                        attn_t[:, do, st * P : (st + 1) * P],
                        wo_t[:, do, n * SC : (n + 1) * SC],
                        start=(do == 0),
                        stop=(do == NM - 1),
                    )
                ob = osbp.tile([P, SC], F32, tag="ob")
                nc.vector.tensor_copy(ob[:], psm[:])
                nc.sync.dma_start(
                    out_r[:, c * 4 + st, n * SC : (n + 1) * SC], ob[:]
                )

            def normalize_pre(pv_e, pv_o):
                """Consume the PV psums right away (frees the banks): copy the
                value rows, gather both denominator rows side by side, and
                compute 1/D = exp(-ln(D)) on ScalarE — Ln and Exp live in the
                same activation table set, so no table reloads, and the
                latency (2 x ~1.2us) hides under the exp stream, unlike the
                3.3us DVE reciprocal which stalled the PE at every boundary."""
                raw_e = nrm.tile([P, SC], F32, tag="raw_e")
                raw_o = nrm.tile([P, SC], F32, tag="raw_o")
                den = nrm.tile([P, 2, SC], F32, tag="den")
                rec = nrm.tile([P, 2, SC], F32, tag="rec")
                nc.vector.tensor_copy(raw_e[0:64, :], pv_e[0:64, :])
                nc.vector.tensor_copy(den[64:65, 0, :], pv_e[64:65, :])
                nc.vector.tensor_copy(raw_o[0:64, :], pv_o[0:64, :])
                nc.vector.tensor_copy(den[64:65, 1, :], pv_o[64:65, :])
                nc.scalar.activation(rec[64:65, :, :], den[64:65, :, :], AF.Ln)
                nc.scalar.activation(den[64:65, :, :], rec[64:65, :, :],
                                     AF.Exp, scale=-1.0)
                return raw_e, raw_o, den

            def normalize_post(raw, recd, j, dst):
                """Broadcast 1/denom across partitions (K=1 matmul), apply."""
                bc = ps_sm.tile([P, SC], F32, tag="sm")
                nc.tensor.matmul(bc[0:64, :], ones_f[64:65, 0:64],
                                 recd[64:65, j, :], start=True, stop=True)
                nc.vector.tensor_mul(dst, bc[0:64, :], raw[0:64, :])

            pending_norm = None   # (raw_e, raw_o, recd, p, attn_t)
            pending_oproj = None  # (c, attn_t)
            pend = []             # deferred PV emissions, depth 2
            for c in range(NCH):
                attn_t = attnp.tile([P, NM, SC], BF16, tag="attn")
                cs = slice(c * SC, (c + 1) * SC)
                for p in range(NM):
                    pv_e = ps_pv.tile([P, SC], F32, tag="pv_e")
                    pv_o = ps_pv.tile([P, SC], F32, tag="pv_o")

                    def emit_pv(ex, t, p=p, pv_e=pv_e, pv_o=pv_o,
                                attn_t=attn_t):
                        for j, (pv, hh) in enumerate(
                            [(pv_e, 2 * p), (pv_o, 2 * p + 1)]
                        ):
                            nc.tensor.matmul(
                                pv[0:65, :],
                                vaug[:, t, hh, 1:66],
                                ex[:, j, :],
                                start=(t == 0),
                                stop=(t == NT - 1),
                            )
                        if t == NT - 1:
                            return (*normalize_pre(pv_e, pv_o), p, attn_t)
                        return None

                    for t in range(NT):
                        sct = ps_sc.tile([P, 2, SC], F32, tag="sc")
                        tw = slice(t * P, (t + 1) * P)
                        nc.tensor.matmul(
                            sct[:, 0, :], kt[0:64, p, tw], qt[0:64, p, cs],
                            start=True, stop=True, tile_position=(0, 0),
                        )
                        nc.tensor.matmul(
                            sct[:, 1, :], kt[64:128, p, tw], qt[64:128, p, cs],
                            start=True, stop=True, tile_position=(64, 0),
                        )
                        ex = expp.tile([P, 2, SC], BF16, tag="ex")
                        nc.scalar.activation(ex[:], sct[:], AF.Exp, scale=0.125)
                        pend.append((ex, t, emit_pv))
                        if len(pend) > 2:
                            ex0, t0, f0 = pend.pop(0)
                            norm = f0(ex0, t0)
                            if norm is not None:
                                pending_norm = norm
                        # -- interleaved off-ceiling work (after the PV pair
                        # so a piece stall can't starve the exp stream) --
                        if pending_norm is not None:
                            ns = 3 if p == 0 else 4
                            if t == ns:
                                raw_e, _, recd, pp, at = pending_norm
                                normalize_post(raw_e, recd, 0, at[0:64, pp, :])
                            elif t == ns + 2:
                                _, raw_o, recd, pp, at = pending_norm
                                normalize_post(raw_o, recd, 1,
                                               at[64:128, pp, :])
                                pending_norm = None
                        if c == 0 and p == 0 and t < 12:
                            g = t // 4 + 1
                            if t == 0:
                                hk[2] = proj_dma(xk_r, 2)
                            elif t == 2:
                                hv[2] = proj_dma(xv_r, 2)
                            elif t == 4:
                                hk[3] = proj_dma(xk_r, 3)
                            elif t == 6:
                                hv[3] = proj_dma(xv_r, 3)
                            proj_chunk(kt, wk_t, xk_r, g, bk_sb,
                                       m_tiles=[t % 4], halves=hk[g])
                            vproj_piece(wv_t, g, t % 4, hv[g])
                        if p == 0 and pending_oproj is not None and \
                                t in (7, 9, 11, 13, 15):
                            oc, oat = pending_oproj
                            i = (t - 7) // 2
                            oproj_piece(oc, oat, i // 2, i % 2)
                        if p == 1 and pending_oproj is not None and \
                                t in (1, 3, 8):
                            oc, oat = pending_oproj
                            i = {1: 5, 3: 6, 8: 7}[t]
                            oproj_piece(oc, oat, i // 2, i % 2)
                            if i == 7:
                                pending_oproj = None
                        if p == 2 and c < NCH - 1 and t == 8:
                            next_halves = proj_dma(xq_r, c + 1)
                        if p == 3 and c < NCH - 1 and t in (2, 8, 11, 14):
                            proj_chunk(qt, wq_t, xq_r, c + 1, bq_sb,
                                       m_tiles=[{2: 0, 8: 1, 11: 2, 14: 3}[t]],
                                       halves=next_halves)

                pending_oproj = (c, attn_t)

            # drain: flush deferred PV pairs, normalize the last head-pair,
            # run the o-projection of the last chunk
            for ex0, t0, f0 in pend:
                norm = f0(ex0, t0)
                if norm is not None:
                    pending_norm = norm
            raw_e, raw_o, recd, pp, at = pending_norm
            normalize_post(raw_e, recd, 0, at[0:64, pp, :])
            normalize_post(raw_o, recd, 1, at[64:128, pp, :])
            oc, oat = pending_oproj
            for st in range(4):
                for n in range(2):
                    oproj_piece(oc, oat, st, n)

    _split_excess_waits(nc)
    return nc


_CACHE = {}


def _get_nc():
    if "nc" not in _CACHE:
        _CACHE["nc"] = build()
    return _CACHE["nc"]


def _f32(x):
    return np.asarray(x).astype(np.float32, copy=False)


def _bf16(x):
    return np.ascontiguousarray(x.astype(bfloat16))


def _prep_core_inputs(c, q, k, v, w_q, b_q, w_k, b_k, w_v, b_v, w_o, b_o):
    b, hg = c // 2, c % 2
    hs = slice(hg * DL, hg * DL + DL)
    return {
        "xq": _bf16(q[b].T),
        "xk": _bf16(k[b].T),
        "xv": _bf16(v[b].T),
        "wq": _bf16(w_q[hs, :].T),
        "wk": _bf16(w_k[hs, :].T),
        "wv": _bf16(w_v[hs, :].T),
        "wo": _bf16(w_o[:, hs].T),
        "bq": np.ascontiguousarray(b_q[hs]),
        "bk": np.ascontiguousarray(b_k[hs]),
        "bv": np.ascontiguousarray(b_v[hs]),
    }


def kernel(q, k, v, w_q, b_q, w_k, b_k, w_v, b_v, w_o, b_o):
    q, k, v = _f32(q), _f32(k), _f32(v)
    w_q, b_q = _f32(w_q), _f32(b_q)
    w_k, b_k = _f32(w_k), _f32(b_k)
    w_v, b_v = _f32(w_v), _f32(b_v)
    w_o, b_o = _f32(w_o), _f32(b_o)

    nc = _get_nc()
    in_maps = [
        _prep_core_inputs(c, q, k, v, w_q, b_q, w_k, b_k, w_v, b_v, w_o, b_o)
        for c in range(8)
    ]
    res = run_bass_kernel_spmd(nc, in_maps, core_ids=list(range(8)))
    out = np.empty((B, S, D), np.float32)
    for b in range(B):
        out[b] = res.results[2 * b]["out"] + res.results[2 * b + 1]["out"] + b_o
    return out


# revision 18
# speedup vs baseline: 1.4849x; 1.0814x over previous
"""Trainium2 Bass SPMD kernel: 16-head MHA (B=4, S=2048, D=1024), fp32 in/out.

Sharding: 8 cores = 4 batches x 2 head-groups (8 heads each). Host pre-
transposes activations to [D, S], pre-slices/transposes weights, and casts
everything (except biases) to bf16 — fp32 matmuls stream at ~1.8 cyc/col on
TRN2 while bf16 streams at 1 cyc/col, and the rel-err budget (2e-2) has
~4x headroom over the measured all-bf16 error (~6e-3).

Pipeline (per s-chunk of 512, per head-pair p):
  - t-loop over 16 key tiles: two K=64 scores matmuls (row-tiled at
    partitions 0/64, run concurrently on the PE) write a 2-bank PSUM tile;
    ScalarE exps it (scale=1/8 folded, no max-subtraction) into bf16 SBUF;
    the PV matmuls for t-1 (65-row V_aug with an appended ones column
    yielding output + softmax denominator) accumulate into 2 PSUM banks.
  - Scores PSUM is double-buffered (2+2 banks) so scores(t+1) overlaps
    exp(t): the ScalarE exp stream (~266us) is the critical engine; all
    other work (o-proj of the previous chunk, q-proj of the next chunk,
    softmax normalization) is sliced into small pieces and slotted into
    the t-loop so the PE rides under the ScalarE ceiling.
  - Normalization: DVE reciprocal_approx_fast on the denominator row
    (~5x faster than exact reciprocal), K=1 ones-matmul broadcast across
    partitions, one DVE multiply into bf16 attn tiles.
  - O-projection contracts attn^T [d_local, s] against w_o columns;
    per-core partial outputs are summed (+b_o) on the host.
"""
import numpy as np
from ml_dtypes import bfloat16

import concourse.bass as bass
import concourse.mybir as mybir
from concourse.tile import TileContext
from concourse.bass_utils import run_bass_kernel_spmd

F32 = mybir.dt.float32
BF16 = mybir.dt.bfloat16
AF = mybir.ActivationFunctionType

B, S, D = 4, 2048, 1024
H, DH = 16, 64
HL = 8        # heads per core
DL = HL * DH  # 512 local model dims
P = 128
SC = 512      # s-chunk width
NCH = S // SC  # 4 s-chunks
ND = D // P    # 8 contraction subtiles for D
NM = DL // P   # 4 m-tiles of local outputs
NT = S // P    # 16 t-tiles

_MAX_WAITS = 1


def _split_excess_waits(nc, max_waits=_MAX_WAITS):
    """walrus here rejects >1 sync-wait per instruction; spill extras onto
    same-engine NoOps inserted before the instruction."""
    f = nc.m.functions[0]
    n = 0
    for bb in f.blocks:
        changed = False
        out = []
        for inst in bb.instructions:
            si = inst.sync_info
            if si is not None and len(si.on_wait) > max_waits:
                waits = list(si.on_wait)
                keep = waits[-max_waits:]
                spill = waits[:-max_waits]
                for i in range(0, len(spill), max_waits):
                    nop = mybir.InstNoOp(name=f"WSPILL-{n}", ins=[], outs=[])
                    n += 1
                    nop.engine = inst.engine
                    nop.sync_info = mybir.SyncInfo(
                        on_wait=spill[i : i + max_waits], on_update=[]
                    )
                    nc.register_instruction(nop, overwrite=True)
                    out.append(nop)
                inst.sync_info = mybir.SyncInfo(
                    on_wait=keep, on_update=list(si.on_update)
                )
                changed = True
            out.append(inst)
        if changed:
            bb.instructions = out
    return n


def build():
    nc = bass.Bass()
    xq = nc.dram_tensor("xq", [D, S], BF16, kind="ExternalInput")
    xk = nc.dram_tensor("xk", [D, S], BF16, kind="ExternalInput")
    xv = nc.dram_tensor("xv", [D, S], BF16, kind="ExternalInput")
    wq = nc.dram_tensor("wq", [D, DL], BF16, kind="ExternalInput")
    wk = nc.dram_tensor("wk", [D, DL], BF16, kind="ExternalInput")
    wv = nc.dram_tensor("wv", [D, DL], BF16, kind="ExternalInput")
    wo = nc.dram_tensor("wo", [DL, D], BF16, kind="ExternalInput")
    bq = nc.dram_tensor("bq", [DL], F32, kind="ExternalInput")
    bk = nc.dram_tensor("bk", [DL], F32, kind="ExternalInput")
    bv = nc.dram_tensor("bv", [DL], F32, kind="ExternalInput")
    out = nc.dram_tensor("out", [S, D], F32, kind="ExternalOutput")

    xq_r = xq.rearrange("(o p) s -> p o s", p=P)
    xk_r = xk.rearrange("(o p) s -> p o s", p=P)
    xv_r = xv.rearrange("(o p) s -> p o s", p=P)
    wq_r = wq.rearrange("(o p) m -> p o m", p=P)
    wk_r = wk.rearrange("(o p) m -> p o m", p=P)
    wv_r = wv.rearrange("(o p) m -> p o m", p=P)
    wo_r = wo.rearrange("(o p) n -> p o n", p=P)
    out_r = out.rearrange("(so p) n -> p so n", p=P)

    with TileContext(nc) as tc:
        with (
            tc.tile_pool(name="persist", bufs=1) as persist,
            tc.tile_pool(name="wpool", bufs=1) as wpool,
            tc.tile_pool(name="xpool", bufs=12) as xpool,
            tc.tile_pool(name="expp", bufs=4) as expp,
            tc.tile_pool(name="attnp", bufs=2) as attnp,
            tc.tile_pool(name="osb", bufs=2) as osbp,
            tc.tile_pool(name="nrm", bufs=2) as nrm,
            tc.tile_pool(name="ps_sc", bufs=2, space="PSUM") as ps_sc,
            tc.tile_pool(name="ps_pv", bufs=1, space="PSUM") as ps_pv,
            tc.tile_pool(name="ps_sm", bufs=2, space="PSUM") as ps_sm,
        ):
            qt = persist.tile([P, NM, S], BF16, tag="qt")
            kt = persist.tile([P, NM, S], BF16, tag="kt")
            vaug = persist.tile([P, NT, HL, 66], BF16, tag="vaug")
            wq_t = persist.tile([P, ND, DL], BF16, tag="wq")
            ones_f = persist.tile([P, P], F32, tag="ones_f")
            bq_sb = persist.tile([P, NM], F32, tag="bq")
            bk_sb = persist.tile([P, NM], F32, tag="bk")
            bv_t = persist.tile([P, DL], F32, tag="bv_t")
            bv_bc = persist.tile([P, DL], F32, tag="bv_bc")

            dens = (
                persist.tile([P, 2, SC], F32, tag="den0", name="den0"),
                persist.tile([P, 2, SC], F32, tag="den1", name="den1"),
            )

            # ---- constants / biases ----
            nc.vector.memset(ones_f[:], 1.0)
            nc.vector.memset(dens[0][64:128, :, :], 0.0)
            nc.vector.memset(dens[1][64:128, :, :], 0.0)
            nc.sync.dma_start(bq_sb[:], bq.rearrange("(o p) -> p o", p=P))
            nc.sync.dma_start(bk_sb[:], bk.rearrange("(o p) -> p o", p=P))
            nc.sync.dma_start(bv_t[0:1, :], bv[None, :])
            ps = ps_sm.tile([P, SC], F32, tag="sm")
            nc.tensor.matmul(ps[:], ones_f[0:1, 0:P], bv_t[0:1, :], start=True,
                             stop=True)
            nc.vector.tensor_copy(bv_bc[:], ps[:])
            # ones column of V_aug (output row 64 = softmax denominator)
            of = ones_f[:, 0:NT * HL].rearrange("p (a b) -> p a b", a=NT)
            nc.vector.tensor_copy(vaug[:, :, :, 65:66], of[:, :, :, None])

            def proj_dma(x_r, c):
                xa = xpool.tile([P, ND // 2, SC], BF16, tag="x")
                xb = xpool.tile([P, ND // 2, SC], BF16, tag="x")
                nc.sync.dma_start(
                    xa[:], x_r[:, 0 : ND // 2, c * SC : (c + 1) * SC])
                nc.sync.dma_start(
                    xb[:], x_r[:, ND // 2 : ND, c * SC : (c + 1) * SC])
                return (xa, xb)

            def proj_chunk(dst, w_tile, x_r, c, bias_sb, m_tiles=range(NM),
                           halves=None):
                """Project one 512-col s-chunk for the given m-tiles."""
                if halves is None:
                    halves = proj_dma(x_r, c)
                for m in m_tiles:
                    psm = ps_sm.tile([P, SC], F32, tag="sm")
                    for k in range(ND):
                        nc.tensor.matmul(
                            psm[:],
                            w_tile[:, k, m * P : (m + 1) * P],
                            halves[k // 4][:, k % 4, :],
                            start=(k == 0),
                            stop=(k == ND - 1),
                        )
                    nc.vector.tensor_add(
                        dst[:, m, c * SC : (c + 1) * SC],
                        psm[:],
                        bias_sb[:, m : m + 1].to_broadcast((P, SC)),
                    )

            def vproj_piece(wv_t, c, i, halves):
                t_o = c * 4 + i
                psm = ps_sm.tile([P, SC], F32, tag="sm")
                for k in range(ND):
                    nc.tensor.matmul(
                        psm[:],
                        halves[k // 4][:, k % 4, i * P : (i + 1) * P],
                        wv_t[:, k, :],
                        start=(k == 0),
                        stop=(k == ND - 1),
                    )
                nc.vector.tensor_add(
                    vaug[:, t_o, :, 1:65],
                    psm[:].rearrange("p (h d) -> p h d", h=HL),
                    bv_bc[:].rearrange("p (h d) -> p h d", h=HL),
                )

            # ---- prefix: chunk-0 projections; K/V chunks 1-3 are folded
            # into the first head-pair's t-loop to keep ScalarE fed early ----
            wk_t = wpool.tile([P, ND, DL], BF16, tag="wk")
            wv_t = wpool.tile([P, ND, DL], BF16, tag="wv")
            wo_t = wpool.tile([P, NM, D], BF16, tag="wo")
            nc.sync.dma_start(wq_t[:], wq_r[:])
            hq0 = proj_dma(xq_r, 0)
            nc.sync.dma_start(wk_t[:], wk_r[:])
            hk = {0: proj_dma(xk_r, 0)}
            nc.sync.dma_start(wv_t[:], wv_r[:])
            hv = {0: proj_dma(xv_r, 0)}
            nc.sync.dma_start(wo_t[:], wo_r[:])
            proj_chunk(qt, wq_t, xq_r, 0, bq_sb, halves=hq0)
            proj_chunk(kt, wk_t, xk_r, 0, bk_sb, halves=hk[0])
            hk[1] = proj_dma(xk_r, 1)
            for i in range(4):
                vproj_piece(wv_t, 0, i, hv[0])
            hv[1] = proj_dma(xv_r, 1)

            # ---- attention + o-proj, per s-chunk ----
            def oproj_piece(c, attn_t, st, n):
                """One [128 s x 512 n] output tile of the o-projection."""
                psm = ps_sm.tile([P, SC], F32, tag="sm")
                for do in range(NM):
                    nc.tensor.matmul(
                        psm[:],
                        attn_t[:, do, st * P : (st + 1) * P],
                        wo_t[:, do, n * SC : (n + 1) * SC],
                        start=(do == 0),
                        stop=(do == NM - 1),
                    )
                ob = osbp.tile([P, SC], F32, tag="ob")
                nc.vector.tensor_copy(ob[:], psm[:])
                nc.sync.dma_start(
                    out_r[:, c * 4 + st, n * SC : (n + 1) * SC], ob[:]
                )

            def normalize_pre(pv_e, pv_o, den):
                """Consume the PV psums right away (frees the banks): copy the
                value rows, gather both denominator rows side by side, and
                compute 1/D = exp(-ln(D)) on ScalarE — Ln and Exp live in the
                same activation table set, so no table reloads, and the
                latency (2 x ~1.2us) hides under the exp stream, unlike the
                3.3us DVE reciprocal which stalled the PE at every boundary.
                `den` rows 65-127 are pre-zeroed so the broadcast matmul can
                run at K=64 — the same PE tiling mode as the scores pairs
                (mode switches drain the PE array)."""
                raw_e = nrm.tile([P, SC], F32, tag="raw_e")
                raw_o = nrm.tile([P, SC], F32, tag="raw_o")
                rec = nrm.tile([P, 2, SC], F32, tag="rec")
                nc.vector.tensor_copy(raw_e[0:64, :], pv_e[0:64, :])
                nc.vector.tensor_copy(den[64:65, 0, :], pv_e[64:65, :])
                nc.vector.tensor_copy(raw_o[0:64, :], pv_o[0:64, :])
                nc.vector.tensor_copy(den[64:65, 1, :], pv_o[64:65, :])
                nc.scalar.activation(rec[64:65, :, :], den[64:65, :, :], AF.Ln)
                nc.scalar.activation(den[64:65, :, :], rec[64:65, :, :],
                                     AF.Exp, scale=-1.0)
                return raw_e, raw_o, den

            def normalize_post(raw, recd, j, dst):
                """Broadcast 1/denom across partitions (K=64 ones-matmul over
                the zero-padded den tile), apply on DVE."""
                bc = ps_sm.tile([P, SC], F32, tag="sm")
                nc.tensor.matmul(bc[:, :], ones_f[64:128, 0:128],
                                 recd[64:128, j, :], start=True, stop=True,
                                 tile_position=(64, 0))
                nc.vector.tensor_mul(dst, bc[0:64, :], raw[0:64, :])

            pending_norm = None   # (raw_e, raw_o, recd, p, attn_t)
            pending_oproj = None  # (c, attn_t)
            pend = []             # deferred PV emissions, depth 2
            for c in range(NCH):
                attn_t = attnp.tile([P, NM, SC], BF16, tag="attn")
                cs = slice(c * SC, (c + 1) * SC)
                for p in range(NM):
                    pv_e = ps_pv.tile([P, SC], F32, tag="pv_e")
                    pv_o = ps_pv.tile([P, SC], F32, tag="pv_o")

                    pidx = c * NM + p

                    def emit_pv(ex, t, p=p, pv_e=pv_e, pv_o=pv_o,
                                attn_t=attn_t, pidx=pidx):
                        for j, (pv, hh) in enumerate(
                            [(pv_e, 2 * p), (pv_o, 2 * p + 1)]
                        ):
                            nc.tensor.matmul(
                                pv[0:65, :],
                                vaug[:, t, hh, 1:66],
                                ex[:, j, :],
                                start=(t == 0),
                                stop=(t == NT - 1),
                            )
                        if t == NT - 1:
                            return (*normalize_pre(pv_e, pv_o,
                                                   dens[pidx % 2]), p, attn_t)
                        return None

                    # two t-tiles per iteration: scores pairs (64-row PE
                    # tiling mode) batched together, then exps, then the
                    # deferred PV pairs + other matmuls (128-row mode) —
                    # tiling-mode switches drain the PE, so batch per mode
                    for tg in range(NT // 2):
                        exs = []
                        if pending_norm is not None and tg in (3, 4):
                            # emitted adjacent to the scores: the broadcast
                            # matmul runs at K=64, same PE mode
                            j = tg - 3
                            raw_e, raw_o, recd, pp, at = pending_norm
                            normalize_post((raw_e, raw_o)[j], recd, j,
                                           at[64 * j:64 * (j + 1), pp, :])
                            if j == 1:
                                pending_norm = None
                        for t in (2 * tg, 2 * tg + 1):
                            sct = ps_sc.tile([P, 2, SC], F32, tag="sc")
                            tw = slice(t * P, (t + 1) * P)
                            nc.tensor.matmul(
                                sct[:, 0, :], kt[0:64, p, tw], qt[0:64, p, cs],
                                start=True, stop=True, tile_position=(0, 0),
                            )
                            nc.tensor.matmul(
                                sct[:, 1, :], kt[64:128, p, tw],
                                qt[64:128, p, cs],
                                start=True, stop=True, tile_position=(64, 0),
                            )
                            ex = expp.tile([P, 2, SC], BF16, tag="ex")
                            nc.scalar.activation(ex[:], sct[:], AF.Exp,
                                                 scale=0.125)
                            exs.append((ex, t, emit_pv))
                        pend.extend(exs)
                        while len(pend) > 2:
                            ex0, t0, f0 = pend.pop(0)
                            norm = f0(ex0, t0)
                            if norm is not None:
                                pending_norm = norm
                        # -- interleaved off-ceiling work (after the PV pairs
                        # so a piece stall can't starve the exp stream) --
                        if c == 0 and p == 0 and tg < 6:
                            for t in (2 * tg, 2 * tg + 1):
                                g = t // 4 + 1
                                if t == 0:
                                    hk[2] = proj_dma(xk_r, 2)
                                elif t == 2:
                                    hv[2] = proj_dma(xv_r, 2)
                                elif t == 4:
                                    hk[3] = proj_dma(xk_r, 3)
                                elif t == 6:
                                    hv[3] = proj_dma(xv_r, 3)
                                proj_chunk(kt, wk_t, xk_r, g, bk_sb,
                                           m_tiles=[t % 4], halves=hk[g])
                                vproj_piece(wv_t, g, t % 4, hv[g])
                        if p == 0 and pending_oproj is not None and \
                                tg in (5, 6, 7):
                            oc, oat = pending_oproj
                            i = tg - 5
                            oproj_piece(oc, oat, i // 2, i % 2)
                        if p == 1 and pending_oproj is not None and \
                                tg in (0, 1, 2, 4, 5):
                            oc, oat = pending_oproj
                            i = {0: 3, 1: 4, 2: 5, 4: 6, 5: 7}[tg]
                            oproj_piece(oc, oat, i // 2, i % 2)
                            if i == 7:
                                pending_oproj = None
                        if p == 2 and c < NCH - 1 and tg == 4:
                            next_halves = proj_dma(xq_r, c + 1)
                        if p == 3 and c < NCH - 1 and tg in (1, 4, 5, 6):
                            proj_chunk(qt, wq_t, xq_r, c + 1, bq_sb,
                                       m_tiles=[{1: 0, 4: 1, 5: 2, 6: 3}[tg]],
                                       halves=next_halves)

                pending_oproj = (c, attn_t)

            # drain: flush deferred PV pairs, normalize the last head-pair,
            # run the o-projection of the last chunk
            for ex0, t0, f0 in pend:
                norm = f0(ex0, t0)
                if norm is not None:
                    pending_norm = norm
            raw_e, raw_o, recd, pp, at = pending_norm
            normalize_post(raw_e, recd, 0, at[0:64, pp, :])
            normalize_post(raw_o, recd, 1, at[64:128, pp, :])
            oc, oat = pending_oproj
            for st in range(4):
                for n in range(2):
                    oproj_piece(oc, oat, st, n)

    _split_excess_waits(nc)
    return nc


_CACHE = {}


def _get_nc():
    if "nc" not in _CACHE:
        _CACHE["nc"] = build()
    return _CACHE["nc"]


def _f32(x):
    return np.asarray(x).astype(np.float32, copy=False)


def _bf16(x):
    return np.ascontiguousarray(x.astype(bfloat16))


def _prep_core_inputs(c, q, k, v, w_q, b_q, w_k, b_k, w_v, b_v, w_o, b_o):
    b, hg = c // 2, c % 2
    hs = slice(hg * DL, hg * DL + DL)
    return {
        "xq": _bf16(q[b].T),
        "xk": _bf16(k[b].T),
        "xv": _bf16(v[b].T),
        "wq": _bf16(w_q[hs, :].T),
        "wk": _bf16(w_k[hs, :].T),
        "wv": _bf16(w_v[hs, :].T),
        "wo": _bf16(w_o[:, hs].T),
        "bq": np.ascontiguousarray(b_q[hs]),
        "bk": np.ascontiguousarray(b_k[hs]),
        "bv": np.ascontiguousarray(b_v[hs]),
    }


def kernel(q, k, v, w_q, b_q, w_k, b_k, w_v, b_v, w_o, b_o):
    q, k, v = _f32(q), _f32(k), _f32(v)
    w_q, b_q = _f32(w_q), _f32(b_q)
    w_k, b_k = _f32(w_k), _f32(b_k)
    w_v, b_v = _f32(w_v), _f32(b_v)
    w_o, b_o = _f32(w_o), _f32(b_o)

    nc = _get_nc()
    in_maps = [
        _prep_core_inputs(c, q, k, v, w_q, b_q, w_k, b_k, w_v, b_v, w_o, b_o)
        for c in range(8)
    ]
    res = run_bass_kernel_spmd(nc, in_maps, core_ids=list(range(8)))
    out = np.empty((B, S, D), np.float32)
    for b in range(B):
        out[b] = res.results[2 * b]["out"] + res.results[2 * b + 1]["out"] + b_o
    return out


# revision 22
# speedup vs baseline: 1.4907x; 1.0039x over previous
"""Trainium2 Bass SPMD kernel: 16-head MHA (B=4, S=2048, D=1024), fp32 in/out.

Sharding: 8 cores = 4 batches x 2 head-groups (8 heads each). Host pre-
transposes activations to [D, S], pre-slices/transposes weights, and casts
everything (except biases) to bf16 — fp32 matmuls stream at ~1.8 cyc/col on
TRN2 while bf16 streams at 1 cyc/col, and the rel-err budget (2e-2) has
~4x headroom over the measured all-bf16 error (~6e-3).

Pipeline (per s-chunk of 512, per head-pair p):
  - t-loop over 16 key tiles: two K=64 scores matmuls (row-tiled at
    partitions 0/64, run concurrently on the PE) write a 2-bank PSUM tile;
    ScalarE exps it (scale=1/8 folded, no max-subtraction) into bf16 SBUF;
    the PV matmuls for t-1 (65-row V_aug with an appended ones column
    yielding output + softmax denominator) accumulate into 2 PSUM banks.
  - Scores PSUM is double-buffered (2+2 banks) so scores(t+1) overlaps
    exp(t): the ScalarE exp stream (~266us) is the critical engine; all
    other work (o-proj of the previous chunk, q-proj of the next chunk,
    softmax normalization) is sliced into small pieces and slotted into
    the t-loop so the PE rides under the ScalarE ceiling.
  - Normalization: DVE reciprocal_approx_fast on the denominator row
    (~5x faster than exact reciprocal), K=1 ones-matmul broadcast across
    partitions, one DVE multiply into bf16 attn tiles.
  - O-projection contracts attn^T [d_local, s] against w_o columns;
    per-core partial outputs are summed (+b_o) on the host.
"""
import numpy as np
from ml_dtypes import bfloat16

import concourse.bass as bass
import concourse.mybir as mybir
from concourse.tile import TileContext
from concourse.bass_utils import run_bass_kernel_spmd

F32 = mybir.dt.float32
BF16 = mybir.dt.bfloat16
AF = mybir.ActivationFunctionType

B, S, D = 4, 2048, 1024
H, DH = 16, 64
HL = 8        # heads per core
DL = HL * DH  # 512 local model dims
P = 128
SC = 512      # s-chunk width
NCH = S // SC  # 4 s-chunks
ND = D // P    # 8 contraction subtiles for D
NM = DL // P   # 4 m-tiles of local outputs
NT = S // P    # 16 t-tiles

_MAX_WAITS = 1


def _split_excess_waits(nc, max_waits=_MAX_WAITS):
    """walrus here rejects >1 sync-wait per instruction; spill extras onto
    same-engine NoOps inserted before the instruction."""
    f = nc.m.functions[0]
    n = 0
    for bb in f.blocks:
        changed = False
        out = []
        for inst in bb.instructions:
            si = inst.sync_info
            if si is not None and len(si.on_wait) > max_waits:
                waits = list(si.on_wait)
                keep = waits[-max_waits:]
                spill = waits[:-max_waits]
                for i in range(0, len(spill), max_waits):
                    nop = mybir.InstNoOp(name=f"WSPILL-{n}", ins=[], outs=[])
                    n += 1
                    nop.engine = inst.engine
                    nop.sync_info = mybir.SyncInfo(
                        on_wait=spill[i : i + max_waits], on_update=[]
                    )
                    nc.register_instruction(nop, overwrite=True)
                    out.append(nop)
                inst.sync_info = mybir.SyncInfo(
                    on_wait=keep, on_update=list(si.on_update)
                )
                changed = True
            out.append(inst)
        if changed:
            bb.instructions = out
    return n


def build():
    nc = bass.Bass()
    xq = nc.dram_tensor("xq", [D, S], BF16, kind="ExternalInput")
    xk = nc.dram_tensor("xk", [D, S], BF16, kind="ExternalInput")
    xv = nc.dram_tensor("xv", [D, S], BF16, kind="ExternalInput")
    wq = nc.dram_tensor("wq", [D, DL], BF16, kind="ExternalInput")
    wk = nc.dram_tensor("wk", [D, DL], BF16, kind="ExternalInput")
    wv = nc.dram_tensor("wv", [D, DL], BF16, kind="ExternalInput")
    wo = nc.dram_tensor("wo", [DL, D], BF16, kind="ExternalInput")
    bq = nc.dram_tensor("bq", [DL], F32, kind="ExternalInput")
    bk = nc.dram_tensor("bk", [DL], F32, kind="ExternalInput")
    bv = nc.dram_tensor("bv", [DL], F32, kind="ExternalInput")
    out = nc.dram_tensor("out", [S, D], F32, kind="ExternalOutput")

    xq_r = xq.rearrange("(o p) s -> p o s", p=P)
    xk_r = xk.rearrange("(o p) s -> p o s", p=P)
    xv_r = xv.rearrange("(o p) s -> p o s", p=P)
    wq_r = wq.rearrange("(o p) m -> p o m", p=P)
    wk_r = wk.rearrange("(o p) m -> p o m", p=P)
    wv_r = wv.rearrange("(o p) m -> p o m", p=P)
    wo_r = wo.rearrange("(o p) n -> p o n", p=P)
    out_r = out.rearrange("(so p) n -> p so n", p=P)

    with TileContext(nc) as tc:
        with (
            tc.tile_pool(name="persist", bufs=1) as persist,
            tc.tile_pool(name="wpool", bufs=1) as wpool,
            tc.tile_pool(name="xpool", bufs=14) as xpool,
            tc.tile_pool(name="expp", bufs=4) as expp,
            tc.tile_pool(name="attnp", bufs=2) as attnp,
            tc.tile_pool(name="osb", bufs=2) as osbp,
            tc.tile_pool(name="nrm", bufs=2) as nrm,
            tc.tile_pool(name="ps_sc", bufs=2, space="PSUM") as ps_sc,
            tc.tile_pool(name="ps_pv", bufs=1, space="PSUM") as ps_pv,
            tc.tile_pool(name="ps_sm", bufs=2, space="PSUM") as ps_sm,
        ):
            qt = persist.tile([P, NM, S], BF16, tag="qt")
            kt = persist.tile([P, NM, S], BF16, tag="kt")
            vaug = persist.tile([P, NT, HL, 66], BF16, tag="vaug")
            wq_t = persist.tile([P, ND, DL], BF16, tag="wq")
            ones_f = persist.tile([P, P], F32, tag="ones_f")
            bq_sb = persist.tile([P, NM], F32, tag="bq")
            bk_sb = persist.tile([P, NM], F32, tag="bk")
            bv_t = persist.tile([P, DL], F32, tag="bv_t")
            bv_bc = persist.tile([P, DL], F32, tag="bv_bc")

            dens = (
                persist.tile([P, 2, SC], F32, tag="den0", name="den0"),
                persist.tile([P, 2, SC], F32, tag="den1", name="den1"),
            )

            # ---- constants / biases ----
            nc.vector.memset(ones_f[:], 1.0)
            nc.vector.memset(dens[0][64:128, :, :], 0.0)
            nc.vector.memset(dens[1][64:128, :, :], 0.0)
            nc.sync.dma_start(bq_sb[:], bq.rearrange("(o p) -> p o", p=P))
            nc.sync.dma_start(bk_sb[:], bk.rearrange("(o p) -> p o", p=P))
            nc.sync.dma_start(bv_t[0:1, :], bv[None, :])
            ps = ps_sm.tile([P, SC], F32, tag="sm")
            nc.tensor.matmul(ps[:], ones_f[0:1, 0:P], bv_t[0:1, :], start=True,
                             stop=True)
            nc.vector.tensor_copy(bv_bc[:], ps[:])
            # ones column of V_aug (output row 64 = softmax denominator)
            of = ones_f[:, 0:NT * HL].rearrange("p (a b) -> p a b", a=NT)
            nc.vector.tensor_copy(vaug[:, :, :, 65:66], of[:, :, :, None])

            def proj_dma(x_r, c):
                xa = xpool.tile([P, ND // 2, SC], BF16, tag="x")
                xb = xpool.tile([P, ND // 2, SC], BF16, tag="x")
                nc.sync.dma_start(
                    xa[:], x_r[:, 0 : ND // 2, c * SC : (c + 1) * SC])
                nc.sync.dma_start(
                    xb[:], x_r[:, ND // 2 : ND, c * SC : (c + 1) * SC])
                return (xa, xb)

            def proj_chunk(dst, w_tile, x_r, c, bias_sb, m_tiles=range(NM),
                           halves=None):
                """Project one 512-col s-chunk for the given m-tiles."""
                if halves is None:
                    halves = proj_dma(x_r, c)
                for m in m_tiles:
                    psm = ps_sm.tile([P, SC], F32, tag="sm")
                    for k in range(ND):
                        nc.tensor.matmul(
                            psm[:],
                            w_tile[:, k, m * P : (m + 1) * P],
                            halves[k // 4][:, k % 4, :],
                            start=(k == 0),
                            stop=(k == ND - 1),
                        )
                    nc.vector.tensor_add(
                        dst[:, m, c * SC : (c + 1) * SC],
                        psm[:],
                        bias_sb[:, m : m + 1].to_broadcast((P, SC)),
                    )

            def vproj_piece(wv_t, c, i, halves):
                t_o = c * 4 + i
                psm = ps_sm.tile([P, SC], F32, tag="sm")
                for k in range(ND):
                    nc.tensor.matmul(
                        psm[:],
                        halves[k // 4][:, k % 4, i * P : (i + 1) * P],
                        wv_t[:, k, :],
                        start=(k == 0),
                        stop=(k == ND - 1),
                    )
                nc.vector.tensor_add(
                    vaug[:, t_o, :, 1:65],
                    psm[:].rearrange("p (h d) -> p h d", h=HL),
                    bv_bc[:].rearrange("p (h d) -> p h d", h=HL),
                )

            # ---- prefix: chunk-0 K/Q projections; V chunk 0 and K/V chunks
            # 1-3 are folded into the first t-loops to keep ScalarE fed ----
            wk_t = wpool.tile([P, ND, DL], BF16, tag="wk")
            wv_t = wpool.tile([P, ND, DL], BF16, tag="wv")
            wo_t = wpool.tile([P, NM, D], BF16, tag="wo")
            nc.sync.dma_start(wk_t[:], wk_r[:])
            nc.scalar.dma_start(wq_t[:], wq_r[:])
            hk = {0: proj_dma(xk_r, 0)}
            hq0 = (
                xpool.tile([P, ND // 2, SC], BF16, tag="x", name="xqa"),
                xpool.tile([P, ND // 2, SC], BF16, tag="x", name="xqb"),
            )
            nc.scalar.dma_start(hq0[0][:], xq_r[:, 0 : ND // 2, 0:SC])
            nc.scalar.dma_start(hq0[1][:], xq_r[:, ND // 2 : ND, 0:SC])
            nc.scalar.dma_start(wv_t[:], wv_r[:])
            hv = {0: proj_dma(xv_r, 0)}
            nc.scalar.dma_start(wo_t[:], wo_r[:])
            proj_chunk(kt, wk_t, xk_r, 0, bk_sb, halves=hk[0])
            proj_chunk(qt, wq_t, xq_r, 0, bq_sb, halves=hq0)
            hk[1] = proj_dma(xk_r, 1)
            hv[1] = proj_dma(xv_r, 1)

            # ---- attention + o-proj, per s-chunk ----
            def oproj_piece(c, attn_t, st, n):
                """One [128 s x 512 n] output tile of the o-projection."""
                psm = ps_sm.tile([P, SC], F32, tag="sm")
                for do in range(NM):
                    nc.tensor.matmul(
                        psm[:],
                        attn_t[:, do, st * P : (st + 1) * P],
                        wo_t[:, do, n * SC : (n + 1) * SC],
                        start=(do == 0),
                        stop=(do == NM - 1),
                    )
                ob = osbp.tile([P, SC], F32, tag="ob")
                nc.vector.tensor_copy(ob[:], psm[:])
                nc.sync.dma_start(
                    out_r[:, c * 4 + st, n * SC : (n + 1) * SC], ob[:]
                )

            def normalize_pre(pv_e, pv_o, den):
                """Consume the PV psums right away (frees the banks): copy the
                value rows, gather both denominator rows side by side, and
                compute 1/D = exp(-ln(D)) on ScalarE — Ln and Exp live in the
                same activation table set, so no table reloads, and the
                latency (2 x ~1.2us) hides under the exp stream, unlike the
                3.3us DVE reciprocal which stalled the PE at every boundary.
                `den` rows 65-127 are pre-zeroed so the broadcast matmul can
                run at K=64 — the same PE tiling mode as the scores pairs
                (mode switches drain the PE array)."""
                raw_e = nrm.tile([P, SC], F32, tag="raw_e")
                raw_o = nrm.tile([P, SC], F32, tag="raw_o")
                rec = nrm.tile([P, 2, SC], F32, tag="rec")
                nc.vector.tensor_copy(raw_e[0:64, :], pv_e[0:64, :])
                nc.vector.tensor_copy(den[64:65, 0, :], pv_e[64:65, :])
                nc.vector.tensor_copy(raw_o[0:64, :], pv_o[0:64, :])
                nc.vector.tensor_copy(den[64:65, 1, :], pv_o[64:65, :])
                nc.scalar.activation(rec[64:65, :, :], den[64:65, :, :], AF.Ln)
                nc.scalar.activation(den[64:65, :, :], rec[64:65, :, :],
                                     AF.Exp, scale=-1.0)
                return raw_e, raw_o, den

            def normalize_post(raw, recd, j, dst):
                """Broadcast 1/denom across partitions (K=64 ones-matmul over
                the zero-padded den tile), apply on DVE."""
                bc = ps_sm.tile([P, SC], F32, tag="sm")
                nc.tensor.matmul(bc[:, :], ones_f[64:128, 0:128],
                                 recd[64:128, j, :], start=True, stop=True,
                                 tile_position=(64, 0))
                nc.vector.tensor_mul(dst, bc[0:64, :], raw[0:64, :])

            pending_norm = None   # (raw_e, raw_o, recd, p, attn_t)
            pending_oproj = None  # (c, attn_t)
            pend = []             # deferred PV emissions, depth 2
            for c in range(NCH):
                attn_t = attnp.tile([P, NM, SC], BF16, tag="attn")
                cs = slice(c * SC, (c + 1) * SC)
                for p in range(NM):
                    pv_e = ps_pv.tile([P, SC], F32, tag="pv_e")
                    pv_o = ps_pv.tile([P, SC], F32, tag="pv_o")

                    pidx = c * NM + p

                    def emit_pv(ex, t, p=p, pv_e=pv_e, pv_o=pv_o,
                                attn_t=attn_t, pidx=pidx):
                        for j, (pv, hh) in enumerate(
                            [(pv_e, 2 * p), (pv_o, 2 * p + 1)]
                        ):
                            nc.tensor.matmul(
                                pv[0:65, :],
                                vaug[:, t, hh, 1:66],
                                ex[:, j, :],
                                start=(t == 0),
                                stop=(t == NT - 1),
                            )
                        if t == NT - 1:
                            return (*normalize_pre(pv_e, pv_o,
                                                   dens[pidx % 2]), p, attn_t)
                        return None

                    # two t-tiles per iteration: scores pairs (64-row PE
                    # tiling mode) batched together, then exps, then the
                    # deferred PV pairs + other matmuls (128-row mode) —
                    # tiling-mode switches drain the PE, so batch per mode
                    for tg in range(NT // 2):
                        exs = []
                        if pending_norm is not None and tg in (3, 4):
                            # emitted adjacent to the scores: the broadcast
                            # matmul runs at K=64, same PE mode
                            j = tg - 3
                            raw_e, raw_o, recd, pp, at = pending_norm
                            normalize_post((raw_e, raw_o)[j], recd, j,
                                           at[64 * j:64 * (j + 1), pp, :])
                            if j == 1:
                                pending_norm = None
                        for t in (2 * tg, 2 * tg + 1):
                            sct = ps_sc.tile([P, 2, SC], F32, tag="sc")
                            tw = slice(t * P, (t + 1) * P)
                            nc.tensor.matmul(
                                sct[:, 0, :], kt[0:64, p, tw], qt[0:64, p, cs],
                                start=True, stop=True, tile_position=(0, 0),
                            )
                            nc.tensor.matmul(
                                sct[:, 1, :], kt[64:128, p, tw],
                                qt[64:128, p, cs],
                                start=True, stop=True, tile_position=(64, 0),
                            )
                            ex = expp.tile([P, 2, SC], BF16, tag="ex")
                            nc.scalar.activation(ex[:], sct[:], AF.Exp,
                                                 scale=0.125)
                            exs.append((ex, t, emit_pv))
                        pend.extend(exs)
                        while len(pend) > 2:
                            ex0, t0, f0 = pend.pop(0)
                            norm = f0(ex0, t0)
                            if norm is not None:
                                pending_norm = norm
                        # -- interleaved off-ceiling work (after the PV pairs
                        # so a piece stall can't starve the exp stream) --
                        if c == 0 and p == 0:
                            # V chunks 0-3 + K chunks 1-3 (m-row 0 only; rows
                            # 1-3 are produced during p1-p3, just before each
                            # head-pair needs them)
                            if tg == 1:
                                hk[2] = proj_dma(xk_r, 2)
                            elif tg == 2:
                                hv[2] = proj_dma(xv_r, 2)
                            elif tg == 3:
                                hk[3] = proj_dma(xk_r, 3)
                            elif tg == 4:
                                hv[3] = proj_dma(xv_r, 3)
                            for vc, vi in {
                                0: [(0, 0), (0, 1)], 1: [(0, 2), (0, 3)],
                                2: [(1, 0), (1, 1)], 3: [(1, 2), (1, 3)],
                                4: [(2, 0), (2, 1)], 5: [(2, 2), (2, 3)],
                                6: [(3, 0), (3, 1), (3, 2)], 7: [(3, 3)],
                            }[tg]:
                                vproj_piece(wv_t, vc, vi, hv[vc])
                            if tg in (1, 3, 5):
                                g = (tg + 1) // 2
                                proj_chunk(kt, wk_t, xk_r, g, bk_sb,
                                           m_tiles=[0], halves=hk[g])
                        if c == 0 and p >= 1 and tg in (0, 1, 2):
                            proj_chunk(kt, wk_t, xk_r, tg + 1, bk_sb,
                                       m_tiles=[p], halves=hk[tg + 1])
                        if p == 0 and pending_oproj is not None and \
                                tg in (5, 6, 7):
                            oc, oat = pending_oproj
                            i = tg - 5
                            oproj_piece(oc, oat, i // 2, i % 2)
                        if p == 1 and pending_oproj is not None and \
                                tg in (0, 1, 2, 4, 5):
                            oc, oat = pending_oproj
                            i = {0: 3, 1: 4, 2: 5, 4: 6, 5: 7}[tg]
                            oproj_piece(oc, oat, i // 2, i % 2)
                            if i == 7:
                                pending_oproj = None
                        if p == 2 and c < NCH - 1 and tg == 4:
                            next_halves = proj_dma(xq_r, c + 1)
                        if p == 3 and c < NCH - 1 and tg in (1, 4, 5, 6):
                            proj_chunk(qt, wq_t, xq_r, c + 1, bq_sb,
                                       m_tiles=[{1: 0, 4: 1, 5: 2, 6: 3}[tg]],
                                       halves=next_halves)

                pending_oproj = (c, attn_t)

            # drain: flush deferred PV pairs, normalize the last head-pair,
            # run the o-projection of the last chunk
            for ex0, t0, f0 in pend:
                norm = f0(ex0, t0)
                if norm is not None:
                    pending_norm = norm
            raw_e, raw_o, recd, pp, at = pending_norm
            normalize_post(raw_e, recd, 0, at[0:64, pp, :])
            normalize_post(raw_o, recd, 1, at[64:128, pp, :])
            oc, oat = pending_oproj
            for st in range(4):
                for n in range(2):
                    oproj_piece(oc, oat, st, n)

    _split_excess_waits(nc)
    return nc


_CACHE = {}


def _get_nc():
    if "nc" not in _CACHE:
        _CACHE["nc"] = build()
    return _CACHE["nc"]


def _f32(x):
    return np.asarray(x).astype(np.float32, copy=False)


def _bf16(x):
    return np.ascontiguousarray(x.astype(bfloat16))


def _prep_core_inputs(c, q, k, v, w_q, b_q, w_k, b_k, w_v, b_v, w_o, b_o):
    b, hg = c // 2, c % 2
    hs = slice(hg * DL, hg * DL + DL)
    return {
        "xq": _bf16(q[b].T),
        "xk": _bf16(k[b].T),
        "xv": _bf16(v[b].T),
        "wq": _bf16(w_q[hs, :].T),
        "wk": _bf16(w_k[hs, :].T),
        "wv": _bf16(w_v[hs, :].T),
        "wo": _bf16(w_o[:, hs].T),
        "bq": np.ascontiguousarray(b_q[hs]),
        "bk": np.ascontiguousarray(b_k[hs]),
        "bv": np.ascontiguousarray(b_v[hs]),
    }


def kernel(q, k, v, w_q, b_q, w_k, b_k, w_v, b_v, w_o, b_o):
    q, k, v = _f32(q), _f32(k), _f32(v)
    w_q, b_q = _f32(w_q), _f32(b_q)
    w_k, b_k = _f32(w_k), _f32(b_k)
    w_v, b_v = _f32(w_v), _f32(b_v)
    w_o, b_o = _f32(w_o), _f32(b_o)

    nc = _get_nc()
    in_maps = [
        _prep_core_inputs(c, q, k, v, w_q, b_q, w_k, b_k, w_v, b_v, w_o, b_o)
        for c in range(8)
    ]
    res = run_bass_kernel_spmd(nc, in_maps, core_ids=list(range(8)))
    out = np.empty((B, S, D), np.float32)
    for b in range(B):
        out[b] = res.results[2 * b]["out"] + res.results[2 * b + 1]["out"] + b_o
    return out


# revision 24
# speedup vs baseline: 1.4978x; 1.0048x over previous
"""Trainium2 Bass SPMD kernel: 16-head MHA (B=4, S=2048, D=1024), fp32 in/out.

Sharding: 8 cores = 4 batches x 2 head-groups (8 heads each). Host pre-
transposes activations to [D, S], pre-slices/transposes weights, and casts
everything (except biases) to bf16 — fp32 matmuls stream at ~1.8 cyc/col on
TRN2 while bf16 streams at 1 cyc/col, and the rel-err budget (2e-2) has
~4x headroom over the measured all-bf16 error (~6e-3).

Pipeline (per s-chunk of 512, per head-pair p), two key tiles per step:
  - Two scores steps: K=64 matmul pairs (row-tiled at partitions 0/64, run
    concurrently on the PE) into double-buffered 2-bank PSUM tiles; batching
    them keeps the PE in one 64-row tiling mode (mode switches drain the
    array). ScalarE exps each tile (scale=1/8 folded, no max-subtraction)
    into bf16 SBUF.
  - The PV matmuls run two steps deferred (65-row V_aug with an appended
    ones column yields output + softmax denominator in 2 PSUM banks), then
    all other 128-row work — o-proj of the previous chunk, q-proj of the
    next chunk, K/V projection pieces during the first head-pair loop — is
    sliced into pieces and slotted in so a piece stall can never starve the
    exp stream, which is co-critical with the PE.
  - Normalization: both denominator rows are gathered side by side and
    1/D = exp(-ln(D)) runs on ScalarE (Ln/Exp share one activation table
    set; the 3.3us DVE reciprocal stalled the PE at every boundary). The
    broadcast is a K=64 ones-matmul over a zero-padded tile (same PE mode),
    deferred several steps so its inputs are always ready.
  - O-projection contracts attn^T [d_local, s] against w_o columns;
    per-core partial outputs are summed (+b_o) on the host.

Measured on HW: 634us (fp32r baseline) -> 425us; rel_absmax err 5.6e-3.
"""
import numpy as np
from ml_dtypes import bfloat16

import concourse.bass as bass
import concourse.mybir as mybir
from concourse.tile import TileContext
from concourse.bass_utils import run_bass_kernel_spmd

F32 = mybir.dt.float32
BF16 = mybir.dt.bfloat16
AF = mybir.ActivationFunctionType

B, S, D = 4, 2048, 1024
H, DH = 16, 64
HL = 8        # heads per core
DL = HL * DH  # 512 local model dims
P = 128
SC = 512      # s-chunk width
NCH = S // SC  # 4 s-chunks
ND = D // P    # 8 contraction subtiles for D
NM = DL // P   # 4 m-tiles of local outputs
NT = S // P    # 16 t-tiles

_MAX_WAITS = 1


def _split_excess_waits(nc, max_waits=_MAX_WAITS):
    """walrus here rejects >1 sync-wait per instruction; spill extras onto
    same-engine NoOps inserted before the instruction."""
    f = nc.m.functions[0]
    n = 0
    for bb in f.blocks:
        changed = False
        out = []
        for inst in bb.instructions:
            si = inst.sync_info
            if si is not None and len(si.on_wait) > max_waits:
                waits = list(si.on_wait)
                keep = waits[-max_waits:]
                spill = waits[:-max_waits]
                for i in range(0, len(spill), max_waits):
                    nop = mybir.InstNoOp(name=f"WSPILL-{n}", ins=[], outs=[])
                    n += 1
                    nop.engine = inst.engine
                    nop.sync_info = mybir.SyncInfo(
                        on_wait=spill[i : i + max_waits], on_update=[]
                    )
                    nc.register_instruction(nop, overwrite=True)
                    out.append(nop)
                inst.sync_info = mybir.SyncInfo(
                    on_wait=keep, on_update=list(si.on_update)
                )
                changed = True
            out.append(inst)
        if changed:
            bb.instructions = out
    return n


def build():
    nc = bass.Bass()
    xq = nc.dram_tensor("xq", [D, S], BF16, kind="ExternalInput")
    xk = nc.dram_tensor("xk", [D, S], BF16, kind="ExternalInput")
    xv = nc.dram_tensor("xv", [D, S], BF16, kind="ExternalInput")
    wq = nc.dram_tensor("wq", [D, DL], BF16, kind="ExternalInput")
    wk = nc.dram_tensor("wk", [D, DL], BF16, kind="ExternalInput")
    wv = nc.dram_tensor("wv", [D, DL], BF16, kind="ExternalInput")
    wo = nc.dram_tensor("wo", [DL, D], BF16, kind="ExternalInput")
    bq = nc.dram_tensor("bq", [DL], F32, kind="ExternalInput")
    bk = nc.dram_tensor("bk", [DL], F32, kind="ExternalInput")
    bv = nc.dram_tensor("bv", [DL], F32, kind="ExternalInput")
    out = nc.dram_tensor("out", [S, D], F32, kind="ExternalOutput")

    xq_r = xq.rearrange("(o p) s -> p o s", p=P)
    xk_r = xk.rearrange("(o p) s -> p o s", p=P)
    xv_r = xv.rearrange("(o p) s -> p o s", p=P)
    wq_r = wq.rearrange("(o p) m -> p o m", p=P)
    wk_r = wk.rearrange("(o p) m -> p o m", p=P)
    wv_r = wv.rearrange("(o p) m -> p o m", p=P)
    wo_r = wo.rearrange("(o p) n -> p o n", p=P)
    out_r = out.rearrange("(so p) n -> p so n", p=P)

    with TileContext(nc) as tc:
        with (
            tc.tile_pool(name="persist", bufs=1) as persist,
            tc.tile_pool(name="wpool", bufs=1) as wpool,
            tc.tile_pool(name="xpool", bufs=14) as xpool,
            tc.tile_pool(name="expp", bufs=6) as expp,
            tc.tile_pool(name="attnp", bufs=2) as attnp,
            tc.tile_pool(name="osb", bufs=2) as osbp,
            tc.tile_pool(name="nrm", bufs=2) as nrm,
            tc.tile_pool(name="ps_sc", bufs=2, space="PSUM") as ps_sc,
            tc.tile_pool(name="ps_pv", bufs=1, space="PSUM") as ps_pv,
            tc.tile_pool(name="ps_sm", bufs=2, space="PSUM") as ps_sm,
        ):
            qt = persist.tile([P, NM, S], BF16, tag="qt")
            kt = persist.tile([P, NM, S], BF16, tag="kt")
            vaug = persist.tile([P, NT, HL, 66], BF16, tag="vaug")
            wq_t = persist.tile([P, ND, DL], BF16, tag="wq")
            ones_f = persist.tile([P, P], F32, tag="ones_f")
            bq_sb = persist.tile([P, NM], F32, tag="bq")
            bk_sb = persist.tile([P, NM], F32, tag="bk")
            bv_t = persist.tile([P, DL], F32, tag="bv_t")
            bv_bc = persist.tile([P, DL], F32, tag="bv_bc")

            dens = (
                persist.tile([P, 2, SC], F32, tag="den0", name="den0"),
                persist.tile([P, 2, SC], F32, tag="den1", name="den1"),
            )

            # ---- constants / biases ----
            nc.vector.memset(ones_f[:], 1.0)
            nc.vector.memset(dens[0][64:128, :, :], 0.0)
            nc.vector.memset(dens[1][64:128, :, :], 0.0)
            nc.sync.dma_start(bq_sb[:], bq.rearrange("(o p) -> p o", p=P))
            nc.sync.dma_start(bk_sb[:], bk.rearrange("(o p) -> p o", p=P))
            nc.sync.dma_start(bv_t[0:1, :], bv[None, :])
            ps = ps_sm.tile([P, SC], F32, tag="sm")
            nc.tensor.matmul(ps[:], ones_f[0:1, 0:P], bv_t[0:1, :], start=True,
                             stop=True)
            nc.vector.tensor_copy(bv_bc[:], ps[:])
            # ones column of V_aug (output row 64 = softmax denominator)
            of = ones_f[:, 0:NT * HL].rearrange("p (a b) -> p a b", a=NT)
            nc.vector.tensor_copy(vaug[:, :, :, 65:66], of[:, :, :, None])

            def proj_dma(x_r, c):
                xa = xpool.tile([P, ND // 2, SC], BF16, tag="x")
                xb = xpool.tile([P, ND // 2, SC], BF16, tag="x")
                nc.sync.dma_start(
                    xa[:], x_r[:, 0 : ND // 2, c * SC : (c + 1) * SC])
                nc.sync.dma_start(
                    xb[:], x_r[:, ND // 2 : ND, c * SC : (c + 1) * SC])
                return (xa, xb)

            def proj_chunk(dst, w_tile, x_r, c, bias_sb, m_tiles=range(NM),
                           halves=None):
                """Project one 512-col s-chunk for the given m-tiles."""
                if halves is None:
                    halves = proj_dma(x_r, c)
                for m in m_tiles:
                    psm = ps_sm.tile([P, SC], F32, tag="sm")
                    for k in range(ND):
                        nc.tensor.matmul(
                            psm[:],
                            w_tile[:, k, m * P : (m + 1) * P],
                            halves[k // 4][:, k % 4, :],
                            start=(k == 0),
                            stop=(k == ND - 1),
                        )
                    nc.vector.tensor_add(
                        dst[:, m, c * SC : (c + 1) * SC],
                        psm[:],
                        bias_sb[:, m : m + 1].to_broadcast((P, SC)),
                    )

            def vproj_piece(wv_t, c, i, halves):
                t_o = c * 4 + i
                psm = ps_sm.tile([P, SC], F32, tag="sm")
                for k in range(ND):
                    nc.tensor.matmul(
                        psm[:],
                        halves[k // 4][:, k % 4, i * P : (i + 1) * P],
                        wv_t[:, k, :],
                        start=(k == 0),
                        stop=(k == ND - 1),
                    )
                nc.vector.tensor_add(
                    vaug[:, t_o, :, 1:65],
                    psm[:].rearrange("p (h d) -> p h d", h=HL),
                    bv_bc[:].rearrange("p (h d) -> p h d", h=HL),
                )

            # ---- prefix: chunk-0 K/Q projections; V chunk 0 and K/V chunks
            # 1-3 are folded into the first t-loops to keep ScalarE fed ----
            wk_t = wpool.tile([P, ND, DL], BF16, tag="wk")
            wv_t = wpool.tile([P, ND, DL], BF16, tag="wv")
            wo_t = wpool.tile([P, NM, D], BF16, tag="wo")
            nc.sync.dma_start(wk_t[:, 0 : ND // 2, :], wk_r[:, 0 : ND // 2, :])
            nc.scalar.dma_start(wq_t[:, 0 : ND // 2, :], wq_r[:, 0 : ND // 2, :])
            nc.sync.dma_start(wk_t[:, ND // 2 :, :], wk_r[:, ND // 2 :, :])
            nc.scalar.dma_start(wq_t[:, ND // 2 :, :], wq_r[:, ND // 2 :, :])
            hk = {0: proj_dma(xk_r, 0)}
            hq0 = (
                xpool.tile([P, ND // 2, SC], BF16, tag="x", name="xqa"),
                xpool.tile([P, ND // 2, SC], BF16, tag="x", name="xqb"),
            )
            nc.scalar.dma_start(hq0[0][:], xq_r[:, 0 : ND // 2, 0:SC])
            nc.scalar.dma_start(hq0[1][:], xq_r[:, ND // 2 : ND, 0:SC])
            nc.scalar.dma_start(wv_t[:], wv_r[:])
            hv = {0: proj_dma(xv_r, 0)}
            nc.scalar.dma_start(wo_t[:], wo_r[:])
            proj_chunk(kt, wk_t, xk_r, 0, bk_sb, halves=hk[0])
            proj_chunk(qt, wq_t, xq_r, 0, bq_sb, halves=hq0)
            hk[1] = proj_dma(xk_r, 1)
            hv[1] = proj_dma(xv_r, 1)

            # ---- attention + o-proj, per s-chunk ----
            def oproj_piece(c, attn_t, st, n):
                """One [128 s x 512 n] output tile of the o-projection."""
                psm = ps_sm.tile([P, SC], F32, tag="sm")
                for do in range(NM):
                    nc.tensor.matmul(
                        psm[:],
                        attn_t[:, do, st * P : (st + 1) * P],
                        wo_t[:, do, n * SC : (n + 1) * SC],
                        start=(do == 0),
                        stop=(do == NM - 1),
                    )
                ob = osbp.tile([P, SC], F32, tag="ob")
                nc.vector.tensor_copy(ob[:], psm[:])
                nc.sync.dma_start(
                    out_r[:, c * 4 + st, n * SC : (n + 1) * SC], ob[:]
                )

            def normalize_pre(pv_e, pv_o, den):
                """Consume the PV psums right away (frees the banks): copy the
                value rows, gather both denominator rows side by side, and
                compute 1/D = exp(-ln(D)) on ScalarE — Ln and Exp live in the
                same activation table set, so no table reloads, and the
                latency (2 x ~1.2us) hides under the exp stream, unlike the
                3.3us DVE reciprocal which stalled the PE at every boundary.
                `den` rows 65-127 are pre-zeroed so the broadcast matmul can
                run at K=64 — the same PE tiling mode as the scores pairs
                (mode switches drain the PE array)."""
                raw_e = nrm.tile([P, SC], F32, tag="raw_e")
                raw_o = nrm.tile([P, SC], F32, tag="raw_o")
                rec = nrm.tile([P, 2, SC], F32, tag="rec")
                nc.vector.tensor_copy(raw_e[0:64, :], pv_e[0:64, :])
                nc.vector.tensor_copy(den[64:65, 0, :], pv_e[64:65, :])
                nc.vector.tensor_copy(raw_o[0:64, :], pv_o[0:64, :])
                nc.vector.tensor_copy(den[64:65, 1, :], pv_o[64:65, :])
                nc.scalar.activation(rec[64:65, :, :], den[64:65, :, :], AF.Ln)
                nc.scalar.activation(den[64:65, :, :], rec[64:65, :, :],
                                     AF.Exp, scale=-1.0)
                return raw_e, raw_o, den

            def normalize_post(raw, recd, j, dst):
                """Broadcast 1/denom across partitions (K=64 ones-matmul over
                the zero-padded den tile), apply on DVE."""
                bc = ps_sm.tile([P, SC], F32, tag="sm")
                nc.tensor.matmul(bc[:, :], ones_f[64:128, 0:128],
                                 recd[64:128, j, :], start=True, stop=True,
                                 tile_position=(64, 0))
                nc.vector.tensor_mul(dst, bc[0:64, :], raw[0:64, :])

            pending_norm = None   # (raw_e, raw_o, recd, p, attn_t)
            pending_oproj = None  # (c, attn_t)
            pend = []             # deferred PV emissions, depth 2
            for c in range(NCH):
                attn_t = attnp.tile([P, NM, SC], BF16, tag="attn")
                cs = slice(c * SC, (c + 1) * SC)
                for p in range(NM):
                    pv_e = ps_pv.tile([P, SC], F32, tag="pv_e")
                    pv_o = ps_pv.tile([P, SC], F32, tag="pv_o")

                    pidx = c * NM + p

                    def emit_pv(ex, t, p=p, pv_e=pv_e, pv_o=pv_o,
                                attn_t=attn_t, pidx=pidx):
                        for j, (pv, hh) in enumerate(
                            [(pv_e, 2 * p), (pv_o, 2 * p + 1)]
                        ):
                            nc.tensor.matmul(
                                pv[0:65, :],
                                vaug[:, t, hh, 1:66],
                                ex[:, j, :],
                                start=(t == 0),
                                stop=(t == NT - 1),
                            )
                        if t == NT - 1:
                            return (*normalize_pre(pv_e, pv_o,
                                                   dens[pidx % 2]), p, attn_t)
                        return None

                    # two t-tiles per iteration: scores pairs (64-row PE
                    # tiling mode) batched together, then exps, then the
                    # deferred PV pairs + other matmuls (128-row mode) —
                    # tiling-mode switches drain the PE, so batch per mode
                    for tg in range(NT // 2):
                        exs = []
                        if pending_norm is not None and tg in (4, 5):
                            # emitted adjacent to the scores: the broadcast
                            # matmul runs at K=64, same PE mode
                            j = tg - 4
                            raw_e, raw_o, recd, pp, at = pending_norm
                            normalize_post((raw_e, raw_o)[j], recd, j,
                                           at[64 * j:64 * (j + 1), pp, :])
                            if j == 1:
                                pending_norm = None
                        for t in (2 * tg, 2 * tg + 1):
                            sct = ps_sc.tile([P, 2, SC], F32, tag="sc")
                            tw = slice(t * P, (t + 1) * P)
                            nc.tensor.matmul(
                                sct[:, 0, :], kt[0:64, p, tw], qt[0:64, p, cs],
                                start=True, stop=True, tile_position=(0, 0),
                            )
                            nc.tensor.matmul(
                                sct[:, 1, :], kt[64:128, p, tw],
                                qt[64:128, p, cs],
                                start=True, stop=True, tile_position=(64, 0),
                            )
                            ex = expp.tile([P, 2, SC], BF16, tag="ex")
                            nc.scalar.activation(ex[:], sct[:], AF.Exp,
                                                 scale=0.125)
                            exs.append((ex, t, emit_pv))
                        pend.extend(exs)
                        while len(pend) > 4:
                            ex0, t0, f0 = pend.pop(0)
                            norm = f0(ex0, t0)
                            if norm is not None:
                                pending_norm = norm
                        # -- interleaved off-ceiling work (after the PV pairs
                        # so a piece stall can't starve the exp stream) --
                        if c == 0 and p == 0:
                            # V chunks 0-3 + K chunks 1-3 (m-row 0 only; rows
                            # 1-3 are produced during p1-p3, just before each
                            # head-pair needs them)
                            if tg == 1:
                                hk[2] = proj_dma(xk_r, 2)
                            elif tg == 2:
                                hv[2] = proj_dma(xv_r, 2)
                            elif tg == 3:
                                hk[3] = proj_dma(xk_r, 3)
                            elif tg == 4:
                                hv[3] = proj_dma(xv_r, 3)
                            for vc, vi in {
                                0: [(0, 0), (0, 1)], 1: [(0, 2), (0, 3)],
                                2: [(1, 0), (1, 1)], 3: [(1, 2), (1, 3)],
                                4: [(2, 0), (2, 1)], 5: [(2, 2), (2, 3)],
                                6: [(3, 0), (3, 1), (3, 2)], 7: [(3, 3)],
                            }[tg]:
                                vproj_piece(wv_t, vc, vi, hv[vc])
                            if tg in (1, 3, 5):
                                g = (tg + 1) // 2
                                proj_chunk(kt, wk_t, xk_r, g, bk_sb,
                                           m_tiles=[0], halves=hk[g])
                        if c == 0 and p >= 1 and tg in (0, 1, 2):
                            proj_chunk(kt, wk_t, xk_r, tg + 1, bk_sb,
                                       m_tiles=[p], halves=hk[tg + 1])
                        if p == 0 and pending_oproj is not None and \
                                tg in (6, 7):
                            oc, oat = pending_oproj
                            i = tg - 6
                            oproj_piece(oc, oat, i // 2, i % 2)
                        if p == 1 and pending_oproj is not None and \
                                tg in (0, 1, 2, 4, 5):
                            oc, oat = pending_oproj
                            i = {0: 2, 1: 3, 2: 4, 4: 5, 5: 6}[tg]
                            oproj_piece(oc, oat, i // 2, i % 2)
                        if p == 2 and pending_oproj is not None and tg == 0:
                            oc, oat = pending_oproj
                            oproj_piece(oc, oat, 3, 1)
                            pending_oproj = None
                        if p == 2 and c < NCH - 1 and tg == 4:
                            next_halves = proj_dma(xq_r, c + 1)
                        if p == 3 and c < NCH - 1 and tg in (1, 4, 5, 6):
                            proj_chunk(qt, wq_t, xq_r, c + 1, bq_sb,
                                       m_tiles=[{1: 0, 4: 1, 5: 2, 6: 3}[tg]],
                                       halves=next_halves)

                pending_oproj = (c, attn_t)

            # drain: flush deferred PV pairs, normalize the last head-pair,
            # run the o-projection of the last chunk
            for ex0, t0, f0 in pend:
                norm = f0(ex0, t0)
                if norm is not None:
                    pending_norm = norm
            raw_e, raw_o, recd, pp, at = pending_norm
            normalize_post(raw_e, recd, 0, at[0:64, pp, :])
            normalize_post(raw_o, recd, 1, at[64:128, pp, :])
            oc, oat = pending_oproj
            for st in range(4):
                for n in range(2):
                    oproj_piece(oc, oat, st, n)

    _split_excess_waits(nc)
    return nc


_CACHE = {}


def _get_nc():
    if "nc" not in _CACHE:
        _CACHE["nc"] = build()
    return _CACHE["nc"]


def _f32(x):
    return np.asarray(x).astype(np.float32, copy=False)


def _bf16(x):
    return np.ascontiguousarray(x.astype(bfloat16))


def _prep_core_inputs(c, q, k, v, w_q, b_q, w_k, b_k, w_v, b_v, w_o, b_o):
    b, hg = c // 2, c % 2
    hs = slice(hg * DL, hg * DL + DL)
    return {
        "xq": _bf16(q[b].T),
        "xk": _bf16(k[b].T),
        "xv": _bf16(v[b].T),
        "wq": _bf16(w_q[hs, :].T),
        "wk": _bf16(w_k[hs, :].T),
        "wv": _bf16(w_v[hs, :].T),
        "wo": _bf16(w_o[:, hs].T),
        "bq": np.ascontiguousarray(b_q[hs]),
        "bk": np.ascontiguousarray(b_k[hs]),
        "bv": np.ascontiguousarray(b_v[hs]),
    }


def kernel(q, k, v, w_q, b_q, w_k, b_k, w_v, b_v, w_o, b_o):
    q, k, v = _f32(q), _f32(k), _f32(v)
    w_q, b_q = _f32(w_q), _f32(b_q)
    w_k, b_k = _f32(w_k), _f32(b_k)
    w_v, b_v = _f32(w_v), _f32(b_v)
    w_o, b_o = _f32(w_o), _f32(b_o)

    nc = _get_nc()
    in_maps = [
        _prep_core_inputs(c, q, k, v, w_q, b_q, w_k, b_k, w_v, b_v, w_o, b_o)
        for c in range(8)
    ]
    res = run_bass_kernel_spmd(nc, in_maps, core_ids=list(range(8)))
    out = np.empty((B, S, D), np.float32)
    for b in range(B):
        out[b] = res.results[2 * b]["out"] + res.results[2 * b + 1]["out"] + b_o
    return out
